# revision 21
# baseline (speedup 1.0000x reference)
"""DenseEquivariantFFT Trainium2 kernel (batch-sharded over 8 cores), v2.

Math: y = IDFT2_cells( sum_{i,s1} DFT2_cells(x)[b,i,s1,f] * KF[o,i,s1,s2,f] ) + bias
where KF = DFT2_cells(kernel[..,mapping]) and f runs over the 64 cell
frequencies in a real (cos/sin) basis.

Device dataflow per core (128 batches), all bf16 with f32 PSUM accum:
 - host pre-transposes x into [(s1-parity, cell), (t, b4, sp, i)] layout,
   so no on-device input transposes are needed.
 - stage C: per batch, one matmul [K=128=(par,c)] x blockdiag(Cf) -> XF
   with partitions (sp,i) and free (batch, parity, fc).
 - stage D: per frequency pair, 4 matmuls [K=128, N=512] against
   deduplicated [kr|ki] weights; re/im recombined on the vector engine
   into an fc-major yf (contiguous writes).
 - stage E: PE transposes (8 per PSUM bank) put (q2,fc) on partitions,
   then one matmul per q-pair against blockdiag(Ci) produces spatial
   output with batch back on partitions; host un-permutes the layout.
"""
import numpy as np
import ml_dtypes

N_CORES = 8
B, CIN, COUT, NS, NCELL, G = 1024, 32, 32, 8, 64, 512
BC = B // N_CORES  # 128 batches per core

_CACHE = {}


def _freq_classes():
    singles, reps = [], []
    for ku in range(8):
        for kv in range(8):
            f = ku * 8 + kv
            cf = ((-ku) % 8) * 8 + ((-kv) % 8)
            if cf == f:
                singles.append(f)
            elif f < cf:
                reps.append(f)
    return singles, reps  # 4, 30


def _transforms():
    singles, reps = _freq_classes()
    u, v = np.meshgrid(np.arange(8), np.arange(8), indexing="ij")

    def theta(f):
        ku, kv = divmod(f, 8)
        return 2 * np.pi * (ku * u + kv * v) / 8

    Cf = np.zeros((64, 64))
    Ci = np.zeros((64, 64))
    for j, f in enumerate(singles):
        Cf[:, j] = np.cos(theta(f)).ravel()
        Ci[j, :] = np.cos(theta(f)).ravel() / 64
    for j, f in enumerate(reps):
        Cf[:, 4 + 2 * j] = np.cos(theta(f)).ravel()
        Cf[:, 5 + 2 * j] = -np.sin(theta(f)).ravel()
        Ci[4 + 2 * j, :] = 2 * np.cos(theta(f)).ravel() / 64
        Ci[5 + 2 * j, :] = -2 * np.sin(theta(f)).ravel() / 64
    return Cf, Ci, singles, reps


def host_constants(kern, bias, mapping):
    """Device weight tensors. W rows use r=(sp,i) with s1=2*sp+h (parity
    halves); W cols use q=(s2,o)."""
    Cf, Ci, singles, reps = _transforms()
    Kexp = kern[:, :, mapping.reshape(NS, NS, NCELL)]  # [o,i,s1,s2,c]
    KF = np.fft.fft2(
        Kexp.reshape(COUT, CIN, NS, NS, 8, 8).astype(np.float64), axes=(-2, -1)
    ).reshape(COUT, CIN, NS, NS, NCELL)

    wp = np.zeros((64, 128, 512), np.float64)  # unit = 2*j + h
    for j, f in enumerate(reps):
        A = KF[..., f]  # [o,i,s1,s2]
        krf = A.real.transpose(2, 1, 3, 0).reshape(NS, CIN, NS * COUT)
        kif = A.imag.transpose(2, 1, 3, 0).reshape(NS, CIN, NS * COUT)
        for h in range(2):
            kr = krf[h::2].reshape(128, 256)
            ki = kif[h::2].reshape(128, 256)
            wp[2 * j + h] = np.concatenate([kr, ki], axis=1)
    ws = np.zeros((8, 128, 256), np.float64)  # unit = 2*js + h
    for js, f in enumerate(singles):
        A = KF[..., f].real.transpose(2, 1, 3, 0).reshape(NS, CIN, NS * COUT)
        for h in range(2):
            ws[2 * js + h] = A[h::2].reshape(128, 256)

    bias_row = 64.0 * np.tile(bias.ravel().astype(np.float64), NS)[None, :]
    bf = ml_dtypes.bfloat16
    return {
        "CfK": np.kron(np.eye(2), Cf).astype(bf),          # [128,128]
        "CiK": np.kron(np.eye(2), Ci).astype(bf),          # [128,128]
        "Wp": np.ascontiguousarray(
            wp.reshape(4, 16, 128, 512).transpose(0, 2, 1, 3)
        ).reshape(4, 128, 16 * 512).astype(bf),
        "Ws": np.ascontiguousarray(
            ws.transpose(1, 0, 2)
        ).reshape(128, 8 * 256).astype(bf),
        "bias_row": bias_row.astype(bf),
        "ones1": np.ones((1, 128), bf),
        "ident": np.eye(128).astype(bf),
    }


def host_prep_x(xc):
    """[128,32,512] f32 -> [128=(par,c), 16384=(t,b4,sp,i)] bf16."""
    xs = xc.reshape(32, 4, CIN, NCELL, 4, 2)  # t,b4,i,c,sp,par
    xt2 = xs.transpose(5, 3, 0, 1, 4, 2).reshape(128, 16384)
    return np.ascontiguousarray(xt2.astype(ml_dtypes.bfloat16))


def host_unpack_y(yo):
    """[128, 16384=(qp,q2,c)] bf16 -> [128, 32, 512] f32; q=(s2,o)=q2*128+qp."""
    arr = np.asarray(yo, np.float32).reshape(BC, 128, 2, 64)   # b, qp, q2, c
    arr = arr.transpose(0, 2, 1, 3).reshape(BC, 256, 64)       # b, q, c
    arr = arr.reshape(BC, NS, COUT, NCELL).transpose(0, 2, 3, 1)  # b,o,c,s2
    return np.ascontiguousarray(arr).reshape(BC, COUT, G)


def host_simulate(x, kern, bias, mapping):
    """f64 numpy mirror of the device algebra (layout validation)."""
    Cf, Ci, singles, reps = _transforms()
    Kexp = kern[:, :, mapping.reshape(NS, NS, NCELL)]
    KF = np.fft.fft2(
        Kexp.reshape(COUT, CIN, NS, NS, 8, 8).astype(np.float64), axes=(-2, -1)
    ).reshape(COUT, CIN, NS, NS, NCELL)
    xs = x.reshape(B, CIN, NCELL, NS).astype(np.float64)
    XF = np.einsum("bics,cf->bisf", xs, Cf)  # [b,i,s1,fc]
    yf = np.zeros((B, NS, COUT, 64))  # [b,s2,o,fc]
    for j, f in enumerate(reps):
        A = KF[..., f]
        yf[..., 4 + 2 * j] = (
            np.einsum("bis,oist->bto", XF[..., 4 + 2 * j], A.real)
            - np.einsum("bis,oist->bto", XF[..., 5 + 2 * j], A.imag)
        )
        yf[..., 5 + 2 * j] = (
            np.einsum("bis,oist->bto", XF[..., 4 + 2 * j], A.imag)
            + np.einsum("bis,oist->bto", XF[..., 5 + 2 * j], A.real)
        )
    for js, f in enumerate(singles):
        yf[..., js] = np.einsum("bis,oist->bto", XF[..., js], KF[..., f].real)
    yf[..., 0] += 64.0 * bias.ravel()[None, None, :]
    y = np.einsum("btof,fc->btoc", yf, Ci)  # [b,s2,o,c]
    y = y.transpose(0, 2, 3, 1).reshape(B, COUT, G)
    return y.astype(np.float32)


def _build_program():
    import concourse.bass as bass
    import concourse.bacc as bacc
    import concourse.mybir as mybir
    from concourse.tile import TileContext

    BF = mybir.dt.bfloat16
    F8 = mybir.dt.float8e4
    F32 = mybir.dt.float32
    nc = bacc.Bacc("TRN2", target_bir_lowering=False, debug=False,
                   num_devices=N_CORES)
    x_d = nc.dram_tensor("x", [128, 16384], BF, kind="ExternalInput")
    cfk_d = nc.dram_tensor("CfK", [128, 128], BF, kind="ExternalInput")
    cik_d = nc.dram_tensor("CiK", [128, 128], BF, kind="ExternalInput")
    wp_d = nc.dram_tensor("Wp", [4, 128, 8192], BF, kind="ExternalInput")
    ws_d = nc.dram_tensor("Ws", [128, 2048], BF, kind="ExternalInput")
    br_d = nc.dram_tensor("bias_row", [1, 256], BF, kind="ExternalInput")
    on_d = nc.dram_tensor("ones1", [1, 128], BF, kind="ExternalInput")
    id_d = nc.dram_tensor("ident", [128, 128], BF, kind="ExternalInput")
    y_d = nc.dram_tensor("y", [128, 16384], BF, kind="ExternalOutput")

    with TileContext(nc) as tc:
        with (
            tc.tile_pool(name="const", bufs=1) as cpool,
            tc.tile_pool(name="xt", bufs=1) as xtpool,
            tc.tile_pool(name="xf3", bufs=1) as xfpool,
            tc.tile_pool(name="w", bufs=1) as wpool,
            tc.tile_pool(name="yf", bufs=1) as yfpool,
            tc.tile_pool(name="yo", bufs=1) as yopool,
            tc.tile_pool(name="pbc", bufs=3) as pbcpool,
            tc.tile_pool(name="yt", bufs=2) as ytpool,
            tc.tile_pool(name="ps", bufs=2, space="PSUM") as pspool,
        ):
            cfk = cpool.tile([128, 128], BF, name="cfk")
            nc.sync.dma_start(out=cfk[:, :], in_=cfk_d[:, :])
            cik = cpool.tile([128, 128], BF, name="cik")
            nc.sync.dma_start(out=cik[:, :], in_=cik_d[:, :])
            br = cpool.tile([1, 256], BF, name="br")
            nc.sync.dma_start(out=br[:, :], in_=br_d[:, :])
            on = cpool.tile([1, 128], BF, name="on")
            nc.sync.dma_start(out=on[:, :], in_=on_d[:, :])
            ident = cpool.tile([128, 128], BF, name="ident")
            nc.sync.dma_start(out=ident[:, :], in_=id_d[:, :])

            # x first (C-stage gates on it), split across both HWDGE rings;
            # weights after, also split.
            xt = [xtpool.tile([128, 4096], BF, name=f"xt{g}", tag=f"xt{g}")
                  for g in range(4)]
            for g in range(4):
                eng = nc.sync if g % 2 == 0 else nc.scalar
                eng.dma_start(
                    out=xt[g][:, :], in_=x_d[:, 4096 * g: 4096 * (g + 1)]
                )
            wsb = [wpool.tile([128, 8192], BF, name=f"wp{k}", tag=f"wp{k}")
                   for k in range(4)]
            for k in range(4):
                eng = nc.sync if k % 2 == 0 else nc.scalar
                eng.dma_start(out=wsb[k][:, :], in_=wp_d[k])
            ws = wpool.tile([128, 2048], BF, name="ws", tag="ws")
            nc.scalar.dma_start(out=ws[:, :], in_=ws_d[:, :])

            xf3 = xfpool.tile([128, 16384], BF, name="xf3")
            yf = yfpool.tile([128, 16384], BF, name="yf")
            yo = yopool.tile([128, 16384], BF, name="yo")

            # ---- stage C: forward cell-DFT, one matmul per batch ----
            for t in range(32):
                g, tl = divmod(t, 8)
                xtr = xt[g][:, :].rearrange(
                    "p (t b4 r) -> p t b4 r", t=8, b4=4
                )
                pc = pspool.tile([128, 512], F32, name="pc", tag="psA")
                for b4 in range(4):
                    nc.tensor.matmul(
                        pc[:, 128 * b4: 128 * b4 + 128],
                        xtr[:, tl, b4, :],
                        cfk[:, :],
                        start=True, stop=True,
                    )
                dst = xf3[:, :].rearrange("p (t q) -> p t q", t=32)
                if t % 2:
                    nc.scalar.copy(dst[:, t, :], pc[:, :])
                else:
                    nc.vector.tensor_copy(dst[:, t, :], pc[:, :])

            # ---- stage D: per-frequency mixing ----
            # yf free = (m, qp) with m = q2*64+fc, q = q2*128+qp: combine
            # writes land as two contiguous 128-runs, and stage-E chunk qp
            # is a single stride-128 run (legal stationary-operand AP).
            xf3r = xf3[:, :].rearrange(
                "p (b s2 fc) -> p s2 fc b", s2=2, fc=64
            )
            yfm = yf[:, :].rearrange(
                "p (q2 fc qp) -> p q2 fc qp", q2=2, fc=64
            )
            for j in range(30):
                k, unit0 = divmod(2 * j, 16)
                pa = pspool.tile([128, 512], F32, name="pa", tag="pa")
                pb = pspool.tile([128, 512], F32, name="pb", tag="pb")
                for h in range(2):
                    rhs = wsb[k][:, 512 * (unit0 + h): 512 * (unit0 + h) + 512]
                    nc.tensor.matmul(
                        pa[:, :], xf3r[:, h, 4 + 2 * j, :], rhs,
                        start=(h == 0), stop=(h == 1),
                    )
                for h in range(2):
                    rhs = wsb[k][:, 512 * (unit0 + h): 512 * (unit0 + h) + 512]
                    nc.tensor.matmul(
                        pb[:, :], xf3r[:, h, 5 + 2 * j, :], rhs,
                        start=(h == 0), stop=(h == 1),
                    )
                pbc = pbcpool.tile([128, 512], BF, name="pbc", tag="pbc")
                nc.scalar.copy(pbc[:, :], pb[:, :])
                fr, fi = 4 + 2 * j, 5 + 2 * j
                par = pa[:, :].rearrange("p (ri q2 qp) -> p ri q2 qp", ri=2, q2=2)
                pbr = pbc[:, :].rearrange("p (ri q2 qp) -> p ri q2 qp", ri=2, q2=2)
                nc.vector.tensor_sub(
                    yfm[:, :, fr, :], par[:, 0], pbr[:, 1]
                )
                nc.vector.tensor_add(
                    yfm[:, :, fi, :], par[:, 1], pbr[:, 0]
                )
            for js in range(4):
                pa = pspool.tile([128, 512], F32, name="pas", tag="pa")
                for h in range(2):
                    rhs = ws[:, 256 * (2 * js + h): 256 * (2 * js + h) + 256]
                    nc.tensor.matmul(
                        pa[:, 0:256], xf3r[:, h, js, :], rhs,
                        start=(h == 0), stop=(h == 1 and js != 0),
                    )
                if js == 0:
                    nc.tensor.matmul(
                        pa[:, 0:256], on[:, :], br[:, :],
                        start=False, stop=True,
                    )
                pasr = pa[:, 0:256].rearrange("p (q2 qp) -> p q2 qp", q2=2)
                nc.vector.tensor_copy(yfm[:, :, js, :], pasr[:, :, :])

            # ---- stage E: PE transpose + inverse cell-DFT ----
            # chunk qp = cols {qp + 128*m'}: single stride-128 run
            yfq = yf[:, :].rearrange("p (m qp) -> p qp m", m=128)
            pe = None
            for grp in range(16):
                pt = pspool.tile([128, 1024], BF, name="pt", tag="psT")
                for k in range(8):
                    qp = 8 * grp + k
                    nc.tensor.transpose(
                        pt[:, 128 * k: 128 * k + 128],
                        yfq[:, qp, :],
                        ident[:, :],
                    )
                yt = ytpool.tile([128, 1024], BF, name="yt", tag="yt")
                if grp % 2:
                    nc.scalar.copy(yt[:, :], pt[:, :])
                else:
                    nc.vector.tensor_copy(yt[:, :], pt[:, :])
                for k in range(8):
                    qp = 8 * grp + k
                    if qp % 4 == 0:
                        pe = pspool.tile(
                            [128, 512], F32, name="pe", tag="psA"
                        )
                    nc.tensor.matmul(
                        pe[:, 128 * (qp % 4): 128 * (qp % 4) + 128],
                        yt[:, 128 * k: 128 * k + 128], cik[:, :],
                        start=True, stop=True,
                    )
                    if qp % 4 == 3:
                        quad = qp // 4
                        dst = yo[:, 512 * quad: 512 * quad + 512]
                        if quad % 2:
                            nc.scalar.copy(dst, pe[:, :])
                        else:
                            nc.vector.tensor_copy(dst, pe[:, :])
                if grp % 2 == 1:
                    blk = grp // 2
                    nc.scalar.dma_start(
                        out=y_d[:, 2048 * blk: 2048 * blk + 2048],
                        in_=yo[:, 2048 * blk: 2048 * blk + 2048],
                    )
    nc.compile()
    return nc


def kernel(**inputs):
    x = np.asarray(inputs["x"], np.float32)
    kern = np.asarray(inputs["kernel"], np.float32)
    bias = np.asarray(inputs["bias"], np.float32)
    mapping = np.asarray(inputs["mapping"])
    from concourse.bass_utils import run_bass_kernel_spmd

    if "nc" not in _CACHE:
        _CACHE["nc"] = _build_program()
    nc = _CACHE["nc"]
    consts = host_constants(kern, bias, mapping)
    in_maps = []
    for c in range(N_CORES):
        m = dict(consts)
        m["x"] = host_prep_x(x[c * BC: (c + 1) * BC])
        in_maps.append(m)
    res = run_bass_kernel_spmd(nc, in_maps, list(range(N_CORES)))
    _CACHE["last_exec_ns"] = res.exec_time_ns
    y = np.concatenate(
        [host_unpack_y(res.results[c]["y"]) for c in range(N_CORES)], 0
    )
    return np.ascontiguousarray(y.astype(np.float32))


# revision 26
# speedup vs baseline: 1.1334x; 1.1334x over previous
"""DenseEquivariantFFT Trainium2 kernel (batch-sharded over 8 cores), v2.

Math: y = IDFT2_cells( sum_{i,s1} DFT2_cells(x)[b,i,s1,f] * KF[o,i,s1,s2,f] ) + bias
where KF = DFT2_cells(kernel[..,mapping]) and f runs over the 64 cell
frequencies in a real (cos/sin) basis.

Device dataflow per core (128 batches), all bf16 with f32 PSUM accum:
 - host pre-transposes x into [(s1-parity, cell), (t, b4, sp, i)] layout,
   so no on-device input transposes are needed.
 - stage C: per batch, one matmul [K=128=(par,c)] x blockdiag(Cf) -> XF
   with partitions (sp,i) and free (batch, parity, fc).
 - stage D: per frequency pair, 4 matmuls [K=128, N=512] against
   deduplicated [kr|ki] weights; re/im recombined on the vector engine
   into an fc-major yf (contiguous writes).
 - stage E: PE transposes (8 per PSUM bank) put (q2,fc) on partitions,
   then one matmul per q-pair against blockdiag(Ci) produces spatial
   output with batch back on partitions; host un-permutes the layout.
"""
import numpy as np
import ml_dtypes

N_CORES = 8
B, CIN, COUT, NS, NCELL, G = 1024, 32, 32, 8, 64, 512
BC = B // N_CORES  # 128 batches per core

_CACHE = {}


def _freq_classes():
    singles, reps = [], []
    for ku in range(8):
        for kv in range(8):
            f = ku * 8 + kv
            cf = ((-ku) % 8) * 8 + ((-kv) % 8)
            if cf == f:
                singles.append(f)
            elif f < cf:
                reps.append(f)
    return singles, reps  # 4, 30


def _transforms():
    singles, reps = _freq_classes()
    u, v = np.meshgrid(np.arange(8), np.arange(8), indexing="ij")

    def theta(f):
        ku, kv = divmod(f, 8)
        return 2 * np.pi * (ku * u + kv * v) / 8

    Cf = np.zeros((64, 64))
    Ci = np.zeros((64, 64))
    for j, f in enumerate(singles):
        Cf[:, j] = np.cos(theta(f)).ravel()
        Ci[j, :] = np.cos(theta(f)).ravel() / 64
    for j, f in enumerate(reps):
        Cf[:, 4 + 2 * j] = np.cos(theta(f)).ravel()
        Cf[:, 5 + 2 * j] = -np.sin(theta(f)).ravel()
        Ci[4 + 2 * j, :] = 2 * np.cos(theta(f)).ravel() / 64
        Ci[5 + 2 * j, :] = -2 * np.sin(theta(f)).ravel() / 64
    return Cf, Ci, singles, reps


def host_constants(kern, bias, mapping):
    """Device weight tensors. W rows use r=(sp,i) with s1=2*sp+h (parity
    halves); W cols use q=(s2,o)."""
    Cf, Ci, singles, reps = _transforms()
    Kexp = kern[:, :, mapping.reshape(NS, NS, NCELL)]  # [o,i,s1,s2,c]
    KF = np.fft.fft2(
        Kexp.reshape(COUT, CIN, NS, NS, 8, 8).astype(np.float64), axes=(-2, -1)
    ).reshape(COUT, CIN, NS, NS, NCELL)

    wp = np.zeros((64, 128, 512), np.float64)  # unit = 2*j + h
    for j, f in enumerate(reps):
        A = KF[..., f]  # [o,i,s1,s2]
        krf = A.real.transpose(2, 1, 3, 0).reshape(NS, CIN, NS * COUT)
        kif = A.imag.transpose(2, 1, 3, 0).reshape(NS, CIN, NS * COUT)
        for h in range(2):
            kr = krf[h::2].reshape(128, 256)
            ki = kif[h::2].reshape(128, 256)
            wp[2 * j + h] = np.concatenate([kr, ki], axis=1)
    ws = np.zeros((8, 128, 256), np.float64)  # unit = 2*js + h
    for js, f in enumerate(singles):
        A = KF[..., f].real.transpose(2, 1, 3, 0).reshape(NS, CIN, NS * COUT)
        for h in range(2):
            ws[2 * js + h] = A[h::2].reshape(128, 256)

    bias_row = 64.0 * np.tile(bias.ravel().astype(np.float64), NS)[None, :]
    bf = ml_dtypes.bfloat16
    return {
        "CfK": np.kron(np.eye(2), Cf).astype(bf),          # [128,128]
        "CiK": np.kron(np.eye(2), Ci).astype(bf),          # [128,128]
        "Wp": np.ascontiguousarray(
            wp.reshape(4, 16, 128, 512).transpose(0, 2, 1, 3)
        ).reshape(4, 128, 16 * 512).astype(bf),
        "Ws": np.ascontiguousarray(
            ws.transpose(1, 0, 2)
        ).reshape(128, 8 * 256).astype(bf),
        "bias_row": bias_row.astype(bf),
        "ones1": np.ones((1, 128), bf),
        "ident": np.eye(128).astype(bf),
    }


def host_prep_x(xc):
    """[128,32,512] f32 -> [128=(par,c), 16384=(t,b4,sp,i)] bf16."""
    xs = xc.reshape(32, 4, CIN, NCELL, 4, 2)  # t,b4,i,c,sp,par
    xt2 = xs.transpose(5, 3, 0, 1, 4, 2).reshape(128, 16384)
    return np.ascontiguousarray(xt2.astype(ml_dtypes.bfloat16))


def host_unpack_y(yo):
    """[128, 16384=(qp,q2,c)] bf16 -> [128, 32, 512] f32; q=(s2,o)=q2*128+qp."""
    arr = np.asarray(yo, np.float32).reshape(BC, 128, 2, 64)   # b, qp, q2, c
    arr = arr.transpose(0, 2, 1, 3).reshape(BC, 256, 64)       # b, q, c
    arr = arr.reshape(BC, NS, COUT, NCELL).transpose(0, 2, 3, 1)  # b,o,c,s2
    return np.ascontiguousarray(arr).reshape(BC, COUT, G)


def host_simulate(x, kern, bias, mapping):
    """f64 numpy mirror of the device algebra (layout validation)."""
    Cf, Ci, singles, reps = _transforms()
    Kexp = kern[:, :, mapping.reshape(NS, NS, NCELL)]
    KF = np.fft.fft2(
        Kexp.reshape(COUT, CIN, NS, NS, 8, 8).astype(np.float64), axes=(-2, -1)
    ).reshape(COUT, CIN, NS, NS, NCELL)
    xs = x.reshape(B, CIN, NCELL, NS).astype(np.float64)
    XF = np.einsum("bics,cf->bisf", xs, Cf)  # [b,i,s1,fc]
    yf = np.zeros((B, NS, COUT, 64))  # [b,s2,o,fc]
    for j, f in enumerate(reps):
        A = KF[..., f]
        yf[..., 4 + 2 * j] = (
            np.einsum("bis,oist->bto", XF[..., 4 + 2 * j], A.real)
            - np.einsum("bis,oist->bto", XF[..., 5 + 2 * j], A.imag)
        )
        yf[..., 5 + 2 * j] = (
            np.einsum("bis,oist->bto", XF[..., 4 + 2 * j], A.imag)
            + np.einsum("bis,oist->bto", XF[..., 5 + 2 * j], A.real)
        )
    for js, f in enumerate(singles):
        yf[..., js] = np.einsum("bis,oist->bto", XF[..., js], KF[..., f].real)
    yf[..., 0] += 64.0 * bias.ravel()[None, None, :]
    y = np.einsum("btof,fc->btoc", yf, Ci)  # [b,s2,o,c]
    y = y.transpose(0, 2, 3, 1).reshape(B, COUT, G)
    return y.astype(np.float32)


def _build_program():
    import concourse.bass as bass
    import concourse.bacc as bacc
    import concourse.mybir as mybir
    from concourse.tile import TileContext

    BF = mybir.dt.bfloat16
    F8 = mybir.dt.float8e4
    F32 = mybir.dt.float32
    nc = bacc.Bacc("TRN2", target_bir_lowering=False, debug=False,
                   num_devices=N_CORES)
    x_d = nc.dram_tensor("x", [128, 16384], BF, kind="ExternalInput")
    cfk_d = nc.dram_tensor("CfK", [128, 128], BF, kind="ExternalInput")
    cik_d = nc.dram_tensor("CiK", [128, 128], BF, kind="ExternalInput")
    wp_d = nc.dram_tensor("Wp", [4, 128, 8192], BF, kind="ExternalInput")
    ws_d = nc.dram_tensor("Ws", [128, 2048], BF, kind="ExternalInput")
    br_d = nc.dram_tensor("bias_row", [1, 256], BF, kind="ExternalInput")
    on_d = nc.dram_tensor("ones1", [1, 128], BF, kind="ExternalInput")
    id_d = nc.dram_tensor("ident", [128, 128], BF, kind="ExternalInput")
    y_d = nc.dram_tensor("y", [128, 16384], BF, kind="ExternalOutput")

    with TileContext(nc) as tc:
        with (
            tc.tile_pool(name="const", bufs=1) as cpool,
            tc.tile_pool(name="xt", bufs=1) as xtpool,
            tc.tile_pool(name="xf3", bufs=1) as xfpool,
            tc.tile_pool(name="w", bufs=1) as wpool,
            tc.tile_pool(name="yf", bufs=1) as yfpool,
            tc.tile_pool(name="yo", bufs=1) as yopool,
            tc.tile_pool(name="pbc", bufs=3) as pbcpool,
            tc.tile_pool(name="yt", bufs=2) as ytpool,
            tc.tile_pool(name="ps", bufs=2, space="PSUM") as pspool,
        ):
            cfk = cpool.tile([128, 128], BF, name="cfk")
            nc.sync.dma_start(out=cfk[:, :], in_=cfk_d[:, :])
            cik = cpool.tile([128, 128], BF, name="cik")
            nc.sync.dma_start(out=cik[:, :], in_=cik_d[:, :])
            br = cpool.tile([1, 256], BF, name="br")
            nc.sync.dma_start(out=br[:, :], in_=br_d[:, :])
            on = cpool.tile([1, 128], BF, name="on")
            nc.sync.dma_start(out=on[:, :], in_=on_d[:, :])
            ident = cpool.tile([128, 128], BF, name="ident")
            nc.sync.dma_start(out=ident[:, :], in_=id_d[:, :])

            # x first (C-stage gates on it), split across both HWDGE rings;
            # weights after, also split.
            xt = [xtpool.tile([128, 2048], BF, name=f"xt{g}", tag=f"xt{g}")
                  for g in range(8)]
            for g in range(8):
                eng = nc.sync if g % 2 == 0 else nc.scalar
                eng.dma_start(
                    out=xt[g][:, :], in_=x_d[:, 2048 * g: 2048 * (g + 1)]
                )
            wsb = [wpool.tile([128, 8192], BF, name=f"wp{k}", tag=f"wp{k}")
                   for k in range(4)]
            for k in range(4):
                eng = nc.sync if k % 2 == 0 else nc.scalar
                eng.dma_start(out=wsb[k][:, :], in_=wp_d[k])
            ws = wpool.tile([128, 2048], BF, name="ws", tag="ws")
            nc.scalar.dma_start(out=ws[:, :], in_=ws_d[:, :])

            xf3 = xfpool.tile([128, 16384], BF, name="xf3")
            yf = yfpool.tile([128, 16384], BF, name="yf")
            yo = yopool.tile([128, 16384], BF, name="yo")

            # ---- stage C: forward cell-DFT, one matmul per batch ----
            # rotate psum across all four same-size tags for a deep pipeline
            ptags = ["psA", "pa", "pb", "psT"]
            for t in range(32):
                g, tl = divmod(t, 4)
                xtr = xt[g][:, :].rearrange(
                    "p (t b4 r) -> p t b4 r", t=4, b4=4
                )
                pc = pspool.tile([128, 512], F32, name="pc", tag=ptags[t % 4])
                for b4 in range(4):
                    nc.tensor.matmul(
                        pc[:, 128 * b4: 128 * b4 + 128],
                        xtr[:, tl, b4, :],
                        cfk[:, :],
                        start=True, stop=True,
                    )
                dst = xf3[:, :].rearrange("p (t q) -> p t q", t=32)
                if t % 2:
                    nc.scalar.copy(dst[:, t, :], pc[:, :])
                else:
                    nc.vector.tensor_copy(dst[:, t, :], pc[:, :])

            # ---- stage D: per-frequency mixing ----
            # yf free = (m, qp) with m = q2*64+fc, q = q2*128+qp: combine
            # writes land as two contiguous 128-runs, and stage-E chunk qp
            # is a single stride-128 run (legal stationary-operand AP).
            xf3r = xf3[:, :].rearrange(
                "p (b s2 fc) -> p s2 fc b", s2=2, fc=64
            )
            yfm = yf[:, :].rearrange(
                "p (q2 fc qp) -> p q2 fc qp", q2=2, fc=64
            )
            for j in range(30):
                k, unit0 = divmod(2 * j, 16)
                ta, tb = ("pa", "pb") if j % 2 == 0 else ("psA", "psT")
                pa = pspool.tile([128, 512], F32, name="pa", tag=ta)
                pb = pspool.tile([128, 512], F32, name="pb", tag=tb)
                for h in range(2):
                    rhs = wsb[k][:, 512 * (unit0 + h): 512 * (unit0 + h) + 512]
                    nc.tensor.matmul(
                        pa[:, :], xf3r[:, h, 4 + 2 * j, :], rhs,
                        start=(h == 0), stop=(h == 1),
                    )
                for h in range(2):
                    rhs = wsb[k][:, 512 * (unit0 + h): 512 * (unit0 + h) + 512]
                    nc.tensor.matmul(
                        pb[:, :], xf3r[:, h, 5 + 2 * j, :], rhs,
                        start=(h == 0), stop=(h == 1),
                    )
                pbc = pbcpool.tile([128, 512], BF, name="pbc", tag="pbc")
                nc.scalar.copy(pbc[:, :], pb[:, :])
                fr, fi = 4 + 2 * j, 5 + 2 * j
                par = pa[:, :].rearrange("p (ri q2 qp) -> p ri q2 qp", ri=2, q2=2)
                pbr = pbc[:, :].rearrange("p (ri q2 qp) -> p ri q2 qp", ri=2, q2=2)
                nc.vector.tensor_sub(
                    yfm[:, :, fr, :], par[:, 0], pbr[:, 1]
                )
                nc.vector.tensor_add(
                    yfm[:, :, fi, :], par[:, 1], pbr[:, 0]
                )
            for js in range(4):
                pa = pspool.tile([128, 512], F32, name="pas", tag=ptags[js])
                for h in range(2):
                    rhs = ws[:, 256 * (2 * js + h): 256 * (2 * js + h) + 256]
                    nc.tensor.matmul(
                        pa[:, 0:256], xf3r[:, h, js, :], rhs,
                        start=(h == 0), stop=(h == 1 and js != 0),
                    )
                if js == 0:
                    nc.tensor.matmul(
                        pa[:, 0:256], on[:, :], br[:, :],
                        start=False, stop=True,
                    )
                pasr = pa[:, 0:256].rearrange("p (q2 qp) -> p q2 qp", q2=2)
                nc.vector.tensor_copy(yfm[:, :, js, :], pasr[:, :, :])

            # ---- stage E: PE transpose + inverse cell-DFT ----
            # chunk qp = cols {qp + 128*m'}: single stride-128 run
            yfq = yf[:, :].rearrange("p (m qp) -> p qp m", m=128)
            pe = None
            for grp in range(16):
                pt = pspool.tile([128, 1024], BF, name="pt", tag="psT")
                for k in range(8):
                    qp = 8 * grp + k
                    nc.tensor.transpose(
                        pt[:, 128 * k: 128 * k + 128],
                        yfq[:, qp, :],
                        ident[:, :],
                    )
                yt = ytpool.tile([128, 1024], BF, name="yt", tag="yt")
                if grp % 2:
                    nc.scalar.copy(yt[:, :], pt[:, :])
                else:
                    nc.vector.tensor_copy(yt[:, :], pt[:, :])
                for k in range(8):
                    qp = 8 * grp + k
                    if qp % 4 == 0:
                        pe = pspool.tile(
                            [128, 512], F32, name="pe",
                            tag=["psA", "pa", "pb"][(qp // 4) % 3],
                        )
                    nc.tensor.matmul(
                        pe[:, 128 * (qp % 4): 128 * (qp % 4) + 128],
                        yt[:, 128 * k: 128 * k + 128], cik[:, :],
                        start=True, stop=True,
                    )
                    if qp % 4 == 3:
                        quad = qp // 4
                        dst = yo[:, 512 * quad: 512 * quad + 512]
                        if quad % 2:
                            nc.scalar.copy(dst, pe[:, :])
                        else:
                            nc.vector.tensor_copy(dst, pe[:, :])
                if grp % 2 == 1:
                    blk = grp // 2
                    nc.scalar.dma_start(
                        out=y_d[:, 2048 * blk: 2048 * blk + 2048],
                        in_=yo[:, 2048 * blk: 2048 * blk + 2048],
                    )
    nc.compile()
    return nc


def kernel(**inputs):
    x = np.asarray(inputs["x"], np.float32)
    kern = np.asarray(inputs["kernel"], np.float32)
    bias = np.asarray(inputs["bias"], np.float32)
    mapping = np.asarray(inputs["mapping"])
    from concourse.bass_utils import run_bass_kernel_spmd

    if "nc" not in _CACHE:
        _CACHE["nc"] = _build_program()
    nc = _CACHE["nc"]
    consts = host_constants(kern, bias, mapping)
    in_maps = []
    for c in range(N_CORES):
        m = dict(consts)
        m["x"] = host_prep_x(x[c * BC: (c + 1) * BC])
        in_maps.append(m)
    res = run_bass_kernel_spmd(nc, in_maps, list(range(N_CORES)))
    _CACHE["last_exec_ns"] = res.exec_time_ns
    y = np.concatenate(
        [host_unpack_y(res.results[c]["y"]) for c in range(N_CORES)], 0
    )
    return np.ascontiguousarray(y.astype(np.float32))


# revision 28
# speedup vs baseline: 1.1752x; 1.0369x over previous
"""DenseEquivariantFFT Trainium2 kernel (batch-sharded over 8 cores), v2.

Math: y = IDFT2_cells( sum_{i,s1} DFT2_cells(x)[b,i,s1,f] * KF[o,i,s1,s2,f] ) + bias
where KF = DFT2_cells(kernel[..,mapping]) and f runs over the 64 cell
frequencies in a real (cos/sin) basis.

Device dataflow per core (128 batches), all bf16 with f32 PSUM accum:
 - host pre-transposes x into [(s1-parity, cell), (t, b4, sp, i)] layout,
   so no on-device input transposes are needed.
 - stage C: per batch, one matmul [K=128=(par,c)] x blockdiag(Cf) -> XF
   with partitions (sp,i) and free (batch, parity, fc).
 - stage D: per frequency pair, 4 matmuls [K=128, N=512] against
   deduplicated [kr|ki] weights; re/im recombined on the vector engine
   into an fc-major yf (contiguous writes).
 - stage E: PE transposes (8 per PSUM bank) put (q2,fc) on partitions,
   then one matmul per q-pair against blockdiag(Ci) produces spatial
   output with batch back on partitions; host un-permutes the layout.
"""
import numpy as np
import ml_dtypes

N_CORES = 8
B, CIN, COUT, NS, NCELL, G = 1024, 32, 32, 8, 64, 512
BC = B // N_CORES  # 128 batches per core

_CACHE = {}


def _freq_classes():
    singles, reps = [], []
    for ku in range(8):
        for kv in range(8):
            f = ku * 8 + kv
            cf = ((-ku) % 8) * 8 + ((-kv) % 8)
            if cf == f:
                singles.append(f)
            elif f < cf:
                reps.append(f)
    return singles, reps  # 4, 30


def _transforms():
    singles, reps = _freq_classes()
    u, v = np.meshgrid(np.arange(8), np.arange(8), indexing="ij")

    def theta(f):
        ku, kv = divmod(f, 8)
        return 2 * np.pi * (ku * u + kv * v) / 8

    Cf = np.zeros((64, 64))
    Ci = np.zeros((64, 64))
    for j, f in enumerate(singles):
        Cf[:, j] = np.cos(theta(f)).ravel()
        Ci[j, :] = np.cos(theta(f)).ravel() / 64
    for j, f in enumerate(reps):
        Cf[:, 4 + 2 * j] = np.cos(theta(f)).ravel()
        Cf[:, 5 + 2 * j] = -np.sin(theta(f)).ravel()
        Ci[4 + 2 * j, :] = 2 * np.cos(theta(f)).ravel() / 64
        Ci[5 + 2 * j, :] = -2 * np.sin(theta(f)).ravel() / 64
    return Cf, Ci, singles, reps


def host_constants(kern, bias, mapping):
    """Device weight tensors. W rows use r=(sp,i) with s1=2*sp+h (parity
    halves); W cols use q=(s2,o)."""
    Cf, Ci, singles, reps = _transforms()
    Kexp = kern[:, :, mapping.reshape(NS, NS, NCELL)]  # [o,i,s1,s2,c]
    KF = np.fft.fft2(
        Kexp.reshape(COUT, CIN, NS, NS, 8, 8).astype(np.float64), axes=(-2, -1)
    ).reshape(COUT, CIN, NS, NS, NCELL)

    wp = np.zeros((64, 128, 512), np.float64)  # unit = 2*j + h
    for j, f in enumerate(reps):
        A = KF[..., f]  # [o,i,s1,s2]
        krf = A.real.transpose(2, 1, 3, 0).reshape(NS, CIN, NS * COUT)
        kif = A.imag.transpose(2, 1, 3, 0).reshape(NS, CIN, NS * COUT)
        for h in range(2):
            kr = krf[h::2].reshape(128, 256)
            ki = kif[h::2].reshape(128, 256)
            wp[2 * j + h] = np.concatenate([kr, ki], axis=1)
    ws = np.zeros((8, 128, 256), np.float64)  # unit = 2*js + h
    for js, f in enumerate(singles):
        A = KF[..., f].real.transpose(2, 1, 3, 0).reshape(NS, CIN, NS * COUT)
        for h in range(2):
            ws[2 * js + h] = A[h::2].reshape(128, 256)

    bias_row = 64.0 * np.tile(bias.ravel().astype(np.float64), NS)[None, :]
    bf = ml_dtypes.bfloat16
    return {
        "CfK": np.kron(np.eye(2), Cf).astype(bf),          # [128,128]
        "CiK": np.kron(np.eye(2), Ci).astype(bf),          # [128,128]
        "Wp": np.ascontiguousarray(
            wp.reshape(4, 16, 128, 512).transpose(0, 2, 1, 3)
        ).reshape(4, 128, 16 * 512).astype(bf),
        "Ws": np.ascontiguousarray(
            ws.transpose(1, 0, 2)
        ).reshape(128, 8 * 256).astype(bf),
        "bias_row": bias_row.astype(bf),
        "ones1": np.ones((1, 128), bf),
        "ident": np.eye(128).astype(bf),
    }


def host_prep_x(xc):
    """[128,32,512] f32 -> [128=(par,c), 16384=(t,b4,sp,i)] bf16."""
    xs = xc.reshape(32, 4, CIN, NCELL, 4, 2)  # t,b4,i,c,sp,par
    xt2 = xs.transpose(5, 3, 0, 1, 4, 2).reshape(128, 16384)
    return np.ascontiguousarray(xt2.astype(ml_dtypes.bfloat16))


def host_unpack_y(yo):
    """[128, 16384=(qp,q2,c)] bf16 -> [128, 32, 512] f32; q=(s2,o)=q2*128+qp."""
    arr = np.asarray(yo, np.float32).reshape(BC, 128, 2, 64)   # b, qp, q2, c
    arr = arr.transpose(0, 2, 1, 3).reshape(BC, 256, 64)       # b, q, c
    arr = arr.reshape(BC, NS, COUT, NCELL).transpose(0, 2, 3, 1)  # b,o,c,s2
    return np.ascontiguousarray(arr).reshape(BC, COUT, G)


def host_simulate(x, kern, bias, mapping):
    """f64 numpy mirror of the device algebra (layout validation)."""
    Cf, Ci, singles, reps = _transforms()
    Kexp = kern[:, :, mapping.reshape(NS, NS, NCELL)]
    KF = np.fft.fft2(
        Kexp.reshape(COUT, CIN, NS, NS, 8, 8).astype(np.float64), axes=(-2, -1)
    ).reshape(COUT, CIN, NS, NS, NCELL)
    xs = x.reshape(B, CIN, NCELL, NS).astype(np.float64)
    XF = np.einsum("bics,cf->bisf", xs, Cf)  # [b,i,s1,fc]
    yf = np.zeros((B, NS, COUT, 64))  # [b,s2,o,fc]
    for j, f in enumerate(reps):
        A = KF[..., f]
        yf[..., 4 + 2 * j] = (
            np.einsum("bis,oist->bto", XF[..., 4 + 2 * j], A.real)
            - np.einsum("bis,oist->bto", XF[..., 5 + 2 * j], A.imag)
        )
        yf[..., 5 + 2 * j] = (
            np.einsum("bis,oist->bto", XF[..., 4 + 2 * j], A.imag)
            + np.einsum("bis,oist->bto", XF[..., 5 + 2 * j], A.real)
        )
    for js, f in enumerate(singles):
        yf[..., js] = np.einsum("bis,oist->bto", XF[..., js], KF[..., f].real)
    yf[..., 0] += 64.0 * bias.ravel()[None, None, :]
    y = np.einsum("btof,fc->btoc", yf, Ci)  # [b,s2,o,c]
    y = y.transpose(0, 2, 3, 1).reshape(B, COUT, G)
    return y.astype(np.float32)


def _build_program():
    import concourse.bass as bass
    import concourse.bacc as bacc
    import concourse.mybir as mybir
    from concourse.tile import TileContext

    BF = mybir.dt.bfloat16
    F8 = mybir.dt.float8e4
    F32 = mybir.dt.float32
    nc = bacc.Bacc("TRN2", target_bir_lowering=False, debug=False,
                   num_devices=N_CORES)
    x_d = nc.dram_tensor("x", [128, 16384], BF, kind="ExternalInput")
    cfk_d = nc.dram_tensor("CfK", [128, 128], BF, kind="ExternalInput")
    cik_d = nc.dram_tensor("CiK", [128, 128], BF, kind="ExternalInput")
    wp_d = nc.dram_tensor("Wp", [4, 128, 8192], BF, kind="ExternalInput")
    ws_d = nc.dram_tensor("Ws", [128, 2048], BF, kind="ExternalInput")
    br_d = nc.dram_tensor("bias_row", [1, 256], BF, kind="ExternalInput")
    on_d = nc.dram_tensor("ones1", [1, 128], BF, kind="ExternalInput")
    id_d = nc.dram_tensor("ident", [128, 128], BF, kind="ExternalInput")
    y_d = nc.dram_tensor("y", [128, 16384], BF, kind="ExternalOutput")

    with TileContext(nc) as tc:
        with (
            tc.tile_pool(name="const", bufs=1) as cpool,
            tc.tile_pool(name="xt", bufs=1) as xtpool,
            tc.tile_pool(name="xf3", bufs=1) as xfpool,
            tc.tile_pool(name="w", bufs=1) as wpool,
            tc.tile_pool(name="yf", bufs=1) as yfpool,
            tc.tile_pool(name="yo", bufs=1) as yopool,
            tc.tile_pool(name="pbc", bufs=3) as pbcpool,
            tc.tile_pool(name="yt", bufs=3) as ytpool,
            tc.tile_pool(name="ps", bufs=2, space="PSUM") as pspool,
        ):
            cfk = cpool.tile([128, 128], BF, name="cfk")
            nc.sync.dma_start(out=cfk[:, :], in_=cfk_d[:, :])
            cik = cpool.tile([128, 128], BF, name="cik")
            nc.sync.dma_start(out=cik[:, :], in_=cik_d[:, :])
            br = cpool.tile([1, 256], BF, name="br")
            nc.sync.dma_start(out=br[:, :], in_=br_d[:, :])
            on = cpool.tile([1, 128], BF, name="on")
            nc.sync.dma_start(out=on[:, :], in_=on_d[:, :])
            ident = cpool.tile([128, 128], BF, name="ident")
            nc.sync.dma_start(out=ident[:, :], in_=id_d[:, :])

            # x first (C-stage gates on it), split across both HWDGE rings;
            # weights after, also split.
            xt = [xtpool.tile([128, 2048], BF, name=f"xt{g}", tag=f"xt{g}")
                  for g in range(8)]
            for g in range(8):
                eng = nc.sync if g % 2 == 0 else nc.scalar
                eng.dma_start(
                    out=xt[g][:, :], in_=x_d[:, 2048 * g: 2048 * (g + 1)]
                )
            wsb = [wpool.tile([128, 8192], BF, name=f"wp{k}", tag=f"wp{k}")
                   for k in range(4)]
            for k in range(4):
                eng = nc.sync if k % 2 == 0 else nc.scalar
                eng.dma_start(out=wsb[k][:, :], in_=wp_d[k])
            ws = wpool.tile([128, 2048], BF, name="ws", tag="ws")
            nc.scalar.dma_start(out=ws[:, :], in_=ws_d[:, :])

            xf3 = xfpool.tile([128, 16384], BF, name="xf3")
            yf = yfpool.tile([128, 16384], BF, name="yf")
            yo = yopool.tile([128, 16384], BF, name="yo")

            # ---- stage C: forward cell-DFT, one matmul per batch ----
            # rotate psum across all four same-size tags for a deep pipeline
            ptags = ["psA", "pa", "pb", "psT"]
            for t in range(32):
                g, tl = divmod(t, 4)
                xtr = xt[g][:, :].rearrange(
                    "p (t b4 r) -> p t b4 r", t=4, b4=4
                )
                pc = pspool.tile([128, 512], F32, name="pc", tag=ptags[t % 4])
                for b4 in range(4):
                    nc.tensor.matmul(
                        pc[:, 128 * b4: 128 * b4 + 128],
                        xtr[:, tl, b4, :],
                        cfk[:, :],
                        start=True, stop=True,
                    )
                dst = xf3[:, :].rearrange("p (t q) -> p t q", t=32)
                if t % 2:
                    nc.scalar.copy(dst[:, t, :], pc[:, :])
                else:
                    nc.vector.tensor_copy(dst[:, t, :], pc[:, :])

            # ---- stage D: per-frequency mixing ----
            # yf free = (m, qp) with m = q2*64+fc, q = q2*128+qp: combine
            # writes land as two contiguous 128-runs, and stage-E chunk qp
            # is a single stride-128 run (legal stationary-operand AP).
            xf3r = xf3[:, :].rearrange(
                "p (b s2 fc) -> p s2 fc b", s2=2, fc=64
            )
            yfm = yf[:, :].rearrange(
                "p (q2 fc qp) -> p q2 fc qp", q2=2, fc=64
            )
            for j in range(30):
                k, unit0 = divmod(2 * j, 16)
                ta, tb = ("pa", "pb") if j % 2 == 0 else ("psA", "psT")
                pa = pspool.tile([128, 512], F32, name="pa", tag=ta)
                pb = pspool.tile([128, 512], F32, name="pb", tag=tb)
                for h in range(2):
                    rhs = wsb[k][:, 512 * (unit0 + h): 512 * (unit0 + h) + 512]
                    nc.tensor.matmul(
                        pa[:, :], xf3r[:, h, 4 + 2 * j, :], rhs,
                        start=(h == 0), stop=(h == 1),
                    )
                for h in range(2):
                    rhs = wsb[k][:, 512 * (unit0 + h): 512 * (unit0 + h) + 512]
                    nc.tensor.matmul(
                        pb[:, :], xf3r[:, h, 5 + 2 * j, :], rhs,
                        start=(h == 0), stop=(h == 1),
                    )
                pbc = pbcpool.tile([128, 512], BF, name="pbc", tag="pbc")
                nc.scalar.copy(pbc[:, :], pb[:, :])
                fr, fi = 4 + 2 * j, 5 + 2 * j
                par = pa[:, :].rearrange("p (ri q2 qp) -> p ri q2 qp", ri=2, q2=2)
                pbr = pbc[:, :].rearrange("p (ri q2 qp) -> p ri q2 qp", ri=2, q2=2)
                nc.vector.tensor_sub(
                    yfm[:, :, fr, :], par[:, 0], pbr[:, 1]
                )
                nc.vector.tensor_add(
                    yfm[:, :, fi, :], par[:, 1], pbr[:, 0]
                )
            for js in range(4):
                pa = pspool.tile([128, 512], F32, name="pas", tag=ptags[js])
                for h in range(2):
                    rhs = ws[:, 256 * (2 * js + h): 256 * (2 * js + h) + 256]
                    nc.tensor.matmul(
                        pa[:, 0:256], xf3r[:, h, js, :], rhs,
                        start=(h == 0), stop=(h == 1 and js != 0),
                    )
                if js == 0:
                    nc.tensor.matmul(
                        pa[:, 0:256], on[:, :], br[:, :],
                        start=False, stop=True,
                    )
                pasr = pa[:, 0:256].rearrange("p (q2 qp) -> p q2 qp", q2=2)
                nc.vector.tensor_copy(yfm[:, :, js, :], pasr[:, :, :])

            # ---- stage E: PE transpose + inverse cell-DFT ----
            # chunk qp = cols {qp + 128*m'}: single stride-128 run
            yfq = yf[:, :].rearrange("p (m qp) -> p qp m", m=128)
            pe = None
            for grp in range(16):
                pt = pspool.tile([128, 1024], BF, name="pt", tag="psT")
                for k in range(8):
                    qp = 8 * grp + k
                    nc.tensor.transpose(
                        pt[:, 128 * k: 128 * k + 128],
                        yfq[:, qp, :],
                        ident[:, :],
                    )
                yt = ytpool.tile([128, 1024], BF, name="yt", tag="yt")
                if grp % 2:
                    nc.scalar.copy(yt[:, :], pt[:, :])
                else:
                    nc.vector.tensor_copy(yt[:, :], pt[:, :])
                for k in range(8):
                    qp = 8 * grp + k
                    if qp % 4 == 0:
                        pe = pspool.tile(
                            [128, 512], F32, name="pe",
                            tag=["psA", "pa", "pb"][(qp // 4) % 3],
                        )
                    nc.tensor.matmul(
                        pe[:, 128 * (qp % 4): 128 * (qp % 4) + 128],
                        yt[:, 128 * k: 128 * k + 128], cik[:, :],
                        start=True, stop=True,
                    )
                    if qp % 4 == 3:
                        quad = qp // 4
                        dst = yo[:, 512 * quad: 512 * quad + 512]
                        if quad % 2:
                            nc.scalar.copy(dst, pe[:, :])
                        else:
                            nc.vector.tensor_copy(dst, pe[:, :])
                if grp % 2 == 1:
                    blk = grp // 2
                    nc.sync.dma_start(
                        out=y_d[:, 2048 * blk: 2048 * blk + 2048],
                        in_=yo[:, 2048 * blk: 2048 * blk + 2048],
                    )
    nc.compile()
    return nc


def kernel(**inputs):
    x = np.asarray(inputs["x"], np.float32)
    kern = np.asarray(inputs["kernel"], np.float32)
    bias = np.asarray(inputs["bias"], np.float32)
    mapping = np.asarray(inputs["mapping"])
    from concourse.bass_utils import run_bass_kernel_spmd

    if "nc" not in _CACHE:
        _CACHE["nc"] = _build_program()
    nc = _CACHE["nc"]
    consts = host_constants(kern, bias, mapping)
    in_maps = []
    for c in range(N_CORES):
        m = dict(consts)
        m["x"] = host_prep_x(x[c * BC: (c + 1) * BC])
        in_maps.append(m)
    res = run_bass_kernel_spmd(nc, in_maps, list(range(N_CORES)))
    _CACHE["last_exec_ns"] = res.exec_time_ns
    y = np.concatenate(
        [host_unpack_y(res.results[c]["y"]) for c in range(N_CORES)], 0
    )
    return np.ascontiguousarray(y.astype(np.float32))


# revision 31
# speedup vs baseline: 1.2551x; 1.0680x over previous
"""DenseEquivariantFFT Trainium2 kernel (batch-sharded over 8 cores), v2.

Math: y = IDFT2_cells( sum_{i,s1} DFT2_cells(x)[b,i,s1,f] * KF[o,i,s1,s2,f] ) + bias
where KF = DFT2_cells(kernel[..,mapping]) and f runs over the 64 cell
frequencies in a real (cos/sin) basis.

Device dataflow per core (128 batches), all bf16 with f32 PSUM accum:
 - host pre-transposes x into [(s1-parity, cell), (t, b4, sp, i)] layout,
   so no on-device input transposes are needed.
 - stage C: per batch, one matmul [K=128=(par,c)] x blockdiag(Cf) -> XF
   with partitions (sp,i) and free (batch, parity, fc).
 - stage D: per frequency pair, 4 matmuls [K=128, N=512] against
   deduplicated [kr|ki] weights; re/im recombined on the vector engine
   into an fc-major yf (contiguous writes).
 - stage E: PE transposes (8 per PSUM bank) put (q2,fc) on partitions,
   then one matmul per q-pair against blockdiag(Ci) produces spatial
   output with batch back on partitions; host un-permutes the layout.
"""
import numpy as np
import ml_dtypes

N_CORES = 8
B, CIN, COUT, NS, NCELL, G = 1024, 32, 32, 8, 64, 512
BC = B // N_CORES  # 128 batches per core

_CACHE = {}


def _freq_classes():
    singles, reps = [], []
    for ku in range(8):
        for kv in range(8):
            f = ku * 8 + kv
            cf = ((-ku) % 8) * 8 + ((-kv) % 8)
            if cf == f:
                singles.append(f)
            elif f < cf:
                reps.append(f)
    return singles, reps  # 4, 30


def _transforms():
    singles, reps = _freq_classes()
    u, v = np.meshgrid(np.arange(8), np.arange(8), indexing="ij")

    def theta(f):
        ku, kv = divmod(f, 8)
        return 2 * np.pi * (ku * u + kv * v) / 8

    Cf = np.zeros((64, 64))
    Ci = np.zeros((64, 64))
    for j, f in enumerate(singles):
        Cf[:, j] = np.cos(theta(f)).ravel()
        Ci[j, :] = np.cos(theta(f)).ravel() / 64
    for j, f in enumerate(reps):
        Cf[:, 4 + 2 * j] = np.cos(theta(f)).ravel()
        Cf[:, 5 + 2 * j] = -np.sin(theta(f)).ravel()
        Ci[4 + 2 * j, :] = 2 * np.cos(theta(f)).ravel() / 64
        Ci[5 + 2 * j, :] = -2 * np.sin(theta(f)).ravel() / 64
    return Cf, Ci, singles, reps


def host_constants(kern, bias, mapping):
    """Device weight tensors. W rows use r=(sp,i) with s1=2*sp+h (parity
    halves); W cols use q=(s2,o)."""
    Cf, Ci, singles, reps = _transforms()
    Kexp = kern[:, :, mapping.reshape(NS, NS, NCELL)]  # [o,i,s1,s2,c]
    KF = np.fft.fft2(
        Kexp.reshape(COUT, CIN, NS, NS, 8, 8).astype(np.float64), axes=(-2, -1)
    ).reshape(COUT, CIN, NS, NS, NCELL)

    wp = np.zeros((64, 128, 512), np.float64)  # unit = 2*j + h
    for j, f in enumerate(reps):
        A = KF[..., f]  # [o,i,s1,s2]
        krf = A.real.transpose(2, 1, 3, 0).reshape(NS, CIN, NS * COUT)
        kif = A.imag.transpose(2, 1, 3, 0).reshape(NS, CIN, NS * COUT)
        for h in range(2):
            kr = krf[h::2].reshape(128, 256)
            ki = kif[h::2].reshape(128, 256)
            wp[2 * j + h] = np.concatenate([kr, ki], axis=1)
    ws = np.zeros((8, 128, 256), np.float64)  # unit = 2*js + h
    for js, f in enumerate(singles):
        A = KF[..., f].real.transpose(2, 1, 3, 0).reshape(NS, CIN, NS * COUT)
        for h in range(2):
            ws[2 * js + h] = A[h::2].reshape(128, 256)

    bias_row = 64.0 * np.tile(bias.ravel().astype(np.float64), NS)[None, :]
    bf = ml_dtypes.bfloat16
    return {
        "CfK": np.kron(np.eye(2), Cf).astype(bf),          # [128,128]
        "CiK": np.kron(np.eye(2), Ci).astype(bf),          # [128,128]
        "Wp": np.ascontiguousarray(
            wp.reshape(4, 16, 128, 512).transpose(0, 2, 1, 3)
        ).reshape(4, 128, 16 * 512).astype(bf),
        "Ws": np.ascontiguousarray(
            ws.transpose(1, 0, 2)
        ).reshape(128, 8 * 256).astype(bf),
        "bias_row": bias_row.astype(bf),
        "ones1": np.ones((1, 128), bf),
        "ident": np.eye(128).astype(bf),
    }


def host_prep_x(xc):
    """[128,32,512] f32 -> [128=(par,c), 16384=(t,b4,sp,i)] bf16."""
    xs = xc.reshape(32, 4, CIN, NCELL, 4, 2)  # t,b4,i,c,sp,par
    xt2 = xs.transpose(5, 3, 0, 1, 4, 2).reshape(128, 16384)
    return np.ascontiguousarray(xt2.astype(ml_dtypes.bfloat16))


def host_unpack_y(yo):
    """[128, 16384=(qp,q2,c)] bf16 -> [128, 32, 512] f32; q=(s2,o)=q2*128+qp."""
    arr = np.asarray(yo, np.float32).reshape(BC, 128, 2, 64)   # b, qp, q2, c
    arr = arr.transpose(0, 2, 1, 3).reshape(BC, 256, 64)       # b, q, c
    arr = arr.reshape(BC, NS, COUT, NCELL).transpose(0, 2, 3, 1)  # b,o,c,s2
    return np.ascontiguousarray(arr).reshape(BC, COUT, G)


def host_simulate(x, kern, bias, mapping):
    """f64 numpy mirror of the device algebra (layout validation)."""
    Cf, Ci, singles, reps = _transforms()
    Kexp = kern[:, :, mapping.reshape(NS, NS, NCELL)]
    KF = np.fft.fft2(
        Kexp.reshape(COUT, CIN, NS, NS, 8, 8).astype(np.float64), axes=(-2, -1)
    ).reshape(COUT, CIN, NS, NS, NCELL)
    xs = x.reshape(B, CIN, NCELL, NS).astype(np.float64)
    XF = np.einsum("bics,cf->bisf", xs, Cf)  # [b,i,s1,fc]
    yf = np.zeros((B, NS, COUT, 64))  # [b,s2,o,fc]
    for j, f in enumerate(reps):
        A = KF[..., f]
        yf[..., 4 + 2 * j] = (
            np.einsum("bis,oist->bto", XF[..., 4 + 2 * j], A.real)
            - np.einsum("bis,oist->bto", XF[..., 5 + 2 * j], A.imag)
        )
        yf[..., 5 + 2 * j] = (
            np.einsum("bis,oist->bto", XF[..., 4 + 2 * j], A.imag)
            + np.einsum("bis,oist->bto", XF[..., 5 + 2 * j], A.real)
        )
    for js, f in enumerate(singles):
        yf[..., js] = np.einsum("bis,oist->bto", XF[..., js], KF[..., f].real)
    yf[..., 0] += 64.0 * bias.ravel()[None, None, :]
    y = np.einsum("btof,fc->btoc", yf, Ci)  # [b,s2,o,c]
    y = y.transpose(0, 2, 3, 1).reshape(B, COUT, G)
    return y.astype(np.float32)


def _build_program():
    import concourse.bass as bass
    import concourse.bacc as bacc
    import concourse.mybir as mybir
    from concourse.tile import TileContext

    BF = mybir.dt.bfloat16
    F8 = mybir.dt.float8e4
    F32 = mybir.dt.float32
    nc = bacc.Bacc("TRN2", target_bir_lowering=False, debug=False,
                   num_devices=N_CORES)
    x_d = nc.dram_tensor("x", [128, 16384], BF, kind="ExternalInput")
    cfk_d = nc.dram_tensor("CfK", [128, 128], BF, kind="ExternalInput")
    cik_d = nc.dram_tensor("CiK", [128, 128], BF, kind="ExternalInput")
    wp_d = nc.dram_tensor("Wp", [4, 128, 8192], BF, kind="ExternalInput")
    ws_d = nc.dram_tensor("Ws", [128, 2048], BF, kind="ExternalInput")
    br_d = nc.dram_tensor("bias_row", [1, 256], BF, kind="ExternalInput")
    on_d = nc.dram_tensor("ones1", [1, 128], BF, kind="ExternalInput")
    id_d = nc.dram_tensor("ident", [128, 128], BF, kind="ExternalInput")
    y_d = nc.dram_tensor("y", [128, 16384], BF, kind="ExternalOutput")

    with TileContext(nc) as tc:
        with (
            tc.tile_pool(name="const", bufs=1) as cpool,
            tc.tile_pool(name="xt", bufs=1) as xtpool,
            tc.tile_pool(name="xf3", bufs=1) as xfpool,
            tc.tile_pool(name="w", bufs=1) as wpool,
            tc.tile_pool(name="yf", bufs=1) as yfpool,
            tc.tile_pool(name="yo", bufs=1) as yopool,
            tc.tile_pool(name="pbc", bufs=3) as pbcpool,
            tc.tile_pool(name="yt", bufs=3) as ytpool,
            tc.tile_pool(name="ps", bufs=2, space="PSUM") as pspool,
        ):
            cfk = cpool.tile([128, 128], BF, name="cfk")
            nc.sync.dma_start(out=cfk[:, :], in_=cfk_d[:, :])
            cik = cpool.tile([128, 128], BF, name="cik")
            nc.sync.dma_start(out=cik[:, :], in_=cik_d[:, :])
            br = cpool.tile([1, 256], BF, name="br")
            nc.sync.dma_start(out=br[:, :], in_=br_d[:, :])
            on = cpool.tile([1, 128], BF, name="on")
            nc.sync.dma_start(out=on[:, :], in_=on_d[:, :])
            ident = cpool.tile([128, 128], BF, name="ident")
            nc.sync.dma_start(out=ident[:, :], in_=id_d[:, :])

            # x first (C-stage gates on it), split across both HWDGE rings;
            # weights after, also split.
            xt = [xtpool.tile([128, 2048], BF, name=f"xt{g}", tag=f"xt{g}")
                  for g in range(8)]
            for g in range(8):
                eng = nc.sync if g % 2 == 0 else nc.scalar
                eng.dma_start(
                    out=xt[g][:, :], in_=x_d[:, 2048 * g: 2048 * (g + 1)]
                )
            # weights in 1MB halves so stage D unblocks as early as possible
            wsb = [wpool.tile([128, 8192], BF, name=f"wp{k}", tag=f"wp{k}")
                   for k in range(4)]
            for k in range(4):
                eng = nc.sync if k % 2 == 0 else nc.scalar
                for hf in range(2):
                    eng.dma_start(
                        out=wsb[k][:, 4096 * hf: 4096 * hf + 4096],
                        in_=wp_d[k][:, 4096 * hf: 4096 * hf + 4096],
                    )
            ws = wpool.tile([128, 2048], BF, name="ws", tag="ws")
            nc.scalar.dma_start(out=ws[:, :], in_=ws_d[:, :])

            xf3 = xfpool.tile([128, 16384], BF, name="xf3")
            yf = yfpool.tile([128, 16384], BF, name="yf")
            yo = yopool.tile([128, 16384], BF, name="yo")

            # ---- stage C: forward cell-DFT, one matmul per batch ----
            # rotate psum across all four same-size tags for a deep pipeline
            ptags = ["psA", "pa", "pb", "psT"]
            for t in range(32):
                g, tl = divmod(t, 4)
                xtr = xt[g][:, :].rearrange(
                    "p (t b4 r) -> p t b4 r", t=4, b4=4
                )
                pc = pspool.tile([128, 512], F32, name="pc", tag=ptags[t % 4])
                for b4 in range(4):
                    nc.tensor.matmul(
                        pc[:, 128 * b4: 128 * b4 + 128],
                        xtr[:, tl, b4, :],
                        cfk[:, :],
                        start=True, stop=True,
                    )
                dst = xf3[:, :].rearrange("p (t q) -> p t q", t=32)
                if t % 2:
                    nc.scalar.copy(dst[:, t, :], pc[:, :])
                else:
                    nc.vector.tensor_copy(dst[:, t, :], pc[:, :])

            # ---- stage D: per-frequency mixing ----
            # yf free = (m, qp) with m = q2*64+fc, q = q2*128+qp: combine
            # writes land as two contiguous 128-runs, and stage-E chunk qp
            # is a single stride-128 run (legal stationary-operand AP).
            xf3r = xf3[:, :].rearrange(
                "p (b s2 fc) -> p s2 fc b", s2=2, fc=64
            )
            yfm = yf[:, :].rearrange(
                "p (q2 fc qp) -> p q2 fc qp", q2=2, fc=64
            )
            for j in range(30):
                k, unit0 = divmod(2 * j, 16)
                ta, tb = ("pa", "pb") if j % 2 == 0 else ("psA", "psT")
                pa = pspool.tile([128, 512], F32, name="pa", tag=ta)
                pb = pspool.tile([128, 512], F32, name="pb", tag=tb)
                for h in range(2):
                    rhs = wsb[k][:, 512 * (unit0 + h): 512 * (unit0 + h) + 512]
                    nc.tensor.matmul(
                        pb[:, :], xf3r[:, h, 5 + 2 * j, :], rhs,
                        start=(h == 0), stop=(h == 1),
                    )
                pbc = pbcpool.tile([128, 512], BF, name="pbc", tag="pbc")
                nc.scalar.copy(pbc[:, :], pb[:, :])
                for h in range(2):
                    rhs = wsb[k][:, 512 * (unit0 + h): 512 * (unit0 + h) + 512]
                    nc.tensor.matmul(
                        pa[:, :], xf3r[:, h, 4 + 2 * j, :], rhs,
                        start=(h == 0), stop=(h == 1),
                    )
                fr, fi = 4 + 2 * j, 5 + 2 * j
                par = pa[:, :].rearrange("p (ri q2 qp) -> p ri q2 qp", ri=2, q2=2)
                pbr = pbc[:, :].rearrange("p (ri q2 qp) -> p ri q2 qp", ri=2, q2=2)
                nc.vector.tensor_sub(
                    yfm[:, :, fr, :], par[:, 0], pbr[:, 1]
                )
                nc.vector.tensor_add(
                    yfm[:, :, fi, :], par[:, 1], pbr[:, 0]
                )
            for js in range(4):
                pa = pspool.tile([128, 512], F32, name="pas", tag=ptags[js])
                for h in range(2):
                    rhs = ws[:, 256 * (2 * js + h): 256 * (2 * js + h) + 256]
                    nc.tensor.matmul(
                        pa[:, 0:256], xf3r[:, h, js, :], rhs,
                        start=(h == 0), stop=(h == 1 and js != 0),
                    )
                if js == 0:
                    nc.tensor.matmul(
                        pa[:, 0:256], on[:, :], br[:, :],
                        start=False, stop=True,
                    )
                pasr = pa[:, 0:256].rearrange("p (q2 qp) -> p q2 qp", q2=2)
                nc.vector.tensor_copy(yfm[:, :, js, :], pasr[:, :, :])

            # ---- stage E: PE transpose + inverse cell-DFT ----
            # chunk qp = cols {qp + 128*m'}: single stride-128 run
            yfq = yf[:, :].rearrange("p (m qp) -> p qp m", m=128)
            pe = None
            for grp in range(16):
                pt = pspool.tile([128, 1024], BF, name="pt", tag="psT")
                for k in range(8):
                    qp = 8 * grp + k
                    nc.tensor.transpose(
                        pt[:, 128 * k: 128 * k + 128],
                        yfq[:, qp, :],
                        ident[:, :],
                    )
                yt = ytpool.tile([128, 1024], BF, name="yt", tag="yt")
                # split the staging copy across both engines to halve the
                # latency in the transpose->matmul chain
                nc.scalar.copy(yt[:, 0:512], pt[:, 0:512])
                nc.vector.tensor_copy(yt[:, 512:1024], pt[:, 512:1024])
                for k in range(8):
                    qp = 8 * grp + k
                    if qp % 4 == 0:
                        pe = pspool.tile(
                            [128, 512], F32, name="pe",
                            tag=["psA", "pa", "pb"][(qp // 4) % 3],
                        )
                    nc.tensor.matmul(
                        pe[:, 128 * (qp % 4): 128 * (qp % 4) + 128],
                        yt[:, 128 * k: 128 * k + 128], cik[:, :],
                        start=True, stop=True,
                    )
                    if qp % 4 == 3:
                        quad = qp // 4
                        dst = yo[:, 512 * quad: 512 * quad + 512]
                        if quad % 2:
                            nc.scalar.copy(dst, pe[:, :])
                        else:
                            nc.vector.tensor_copy(dst, pe[:, :])
                if grp % 2 == 1:
                    blk = grp // 2
                    nc.sync.dma_start(
                        out=y_d[:, 2048 * blk: 2048 * blk + 2048],
                        in_=yo[:, 2048 * blk: 2048 * blk + 2048],
                    )
    nc.compile()
    return nc


def kernel(**inputs):
    x = np.asarray(inputs["x"], np.float32)
    kern = np.asarray(inputs["kernel"], np.float32)
    bias = np.asarray(inputs["bias"], np.float32)
    mapping = np.asarray(inputs["mapping"])
    from concourse.bass_utils import run_bass_kernel_spmd

    if "nc" not in _CACHE:
        _CACHE["nc"] = _build_program()
    nc = _CACHE["nc"]
    consts = host_constants(kern, bias, mapping)
    in_maps = []
    for c in range(N_CORES):
        m = dict(consts)
        m["x"] = host_prep_x(x[c * BC: (c + 1) * BC])
        in_maps.append(m)
    res = run_bass_kernel_spmd(nc, in_maps, list(range(N_CORES)))
    _CACHE["last_exec_ns"] = res.exec_time_ns
    y = np.concatenate(
        [host_unpack_y(res.results[c]["y"]) for c in range(N_CORES)], 0
    )
    return np.ascontiguousarray(y.astype(np.float32))


# revision 32
# speedup vs baseline: 1.3653x; 1.0878x over previous
"""DenseEquivariantFFT Trainium2 kernel (batch-sharded over 8 cores), v2.

Math: y = IDFT2_cells( sum_{i,s1} DFT2_cells(x)[b,i,s1,f] * KF[o,i,s1,s2,f] ) + bias
where KF = DFT2_cells(kernel[..,mapping]) and f runs over the 64 cell
frequencies in a real (cos/sin) basis.

Device dataflow per core (128 batches), all bf16 with f32 PSUM accum:
 - host pre-transposes x into [(s1-parity, cell), (t, b4, sp, i)] layout,
   so no on-device input transposes are needed.
 - stage C: per batch, one matmul [K=128=(par,c)] x blockdiag(Cf) -> XF
   with partitions (sp,i) and free (batch, parity, fc).
 - stage D: per frequency pair, 4 matmuls [K=128, N=512] against
   deduplicated [kr|ki] weights; re/im recombined on the vector engine
   into an fc-major yf (contiguous writes).
 - stage E: PE transposes (8 per PSUM bank) put (q2,fc) on partitions,
   then one matmul per q-pair against blockdiag(Ci) produces spatial
   output with batch back on partitions; host un-permutes the layout.
"""
import numpy as np
import ml_dtypes

N_CORES = 8
B, CIN, COUT, NS, NCELL, G = 1024, 32, 32, 8, 64, 512
BC = B // N_CORES  # 128 batches per core

_CACHE = {}


def _freq_classes():
    singles, reps = [], []
    for ku in range(8):
        for kv in range(8):
            f = ku * 8 + kv
            cf = ((-ku) % 8) * 8 + ((-kv) % 8)
            if cf == f:
                singles.append(f)
            elif f < cf:
                reps.append(f)
    return singles, reps  # 4, 30


def _transforms():
    singles, reps = _freq_classes()
    u, v = np.meshgrid(np.arange(8), np.arange(8), indexing="ij")

    def theta(f):
        ku, kv = divmod(f, 8)
        return 2 * np.pi * (ku * u + kv * v) / 8

    Cf = np.zeros((64, 64))
    Ci = np.zeros((64, 64))
    for j, f in enumerate(singles):
        Cf[:, j] = np.cos(theta(f)).ravel()
        Ci[j, :] = np.cos(theta(f)).ravel() / 64
    for j, f in enumerate(reps):
        Cf[:, 4 + 2 * j] = np.cos(theta(f)).ravel()
        Cf[:, 5 + 2 * j] = -np.sin(theta(f)).ravel()
        Ci[4 + 2 * j, :] = 2 * np.cos(theta(f)).ravel() / 64
        Ci[5 + 2 * j, :] = -2 * np.sin(theta(f)).ravel() / 64
    return Cf, Ci, singles, reps


def host_constants(kern, bias, mapping):
    """Device weight tensors. W rows use r=(sp,i) with s1=2*sp+h (parity
    halves); W cols use q=(s2,o)."""
    Cf, Ci, singles, reps = _transforms()
    Kexp = kern[:, :, mapping.reshape(NS, NS, NCELL)]  # [o,i,s1,s2,c]
    KF = np.fft.fft2(
        Kexp.reshape(COUT, CIN, NS, NS, 8, 8).astype(np.float64), axes=(-2, -1)
    ).reshape(COUT, CIN, NS, NS, NCELL)

    wp = np.zeros((64, 128, 512), np.float64)  # unit = 2*j + h
    for j, f in enumerate(reps):
        A = KF[..., f]  # [o,i,s1,s2]
        krf = A.real.transpose(2, 1, 3, 0).reshape(NS, CIN, NS * COUT)
        kif = A.imag.transpose(2, 1, 3, 0).reshape(NS, CIN, NS * COUT)
        for h in range(2):
            kr = krf[h::2].reshape(128, 256)
            ki = kif[h::2].reshape(128, 256)
            wp[2 * j + h] = np.concatenate([kr, ki], axis=1)
    ws = np.zeros((8, 128, 256), np.float64)  # unit = 2*js + h
    for js, f in enumerate(singles):
        A = KF[..., f].real.transpose(2, 1, 3, 0).reshape(NS, CIN, NS * COUT)
        for h in range(2):
            ws[2 * js + h] = A[h::2].reshape(128, 256)

    bias_row = 64.0 * np.tile(bias.ravel().astype(np.float64), NS)[None, :]
    bf = ml_dtypes.bfloat16
    return {
        "CfK": np.kron(np.eye(2), Cf).astype(bf),          # [128,128]
        "CiK": np.kron(np.eye(2), Ci).astype(bf),          # [128,128]
        "Wp": np.ascontiguousarray(
            wp.reshape(4, 16, 128, 512).transpose(0, 2, 1, 3)
        ).reshape(4, 128, 16 * 512).astype(bf),
        "Ws": np.ascontiguousarray(
            ws.transpose(1, 0, 2)
        ).reshape(128, 8 * 256).astype(bf),
        "bias_row": bias_row.astype(bf),
        "ones1": np.ones((1, 128), bf),
        "ident": np.eye(128).astype(bf),
    }


def host_prep_x(xc):
    """[128,32,512] f32 -> [128=(par,c), 16384=(t,b4,sp,i)] bf16."""
    xs = xc.reshape(32, 4, CIN, NCELL, 4, 2)  # t,b4,i,c,sp,par
    xt2 = xs.transpose(5, 3, 0, 1, 4, 2).reshape(128, 16384)
    return np.ascontiguousarray(xt2.astype(ml_dtypes.bfloat16))


def host_unpack_y(yo):
    """[128, 16384=(qp,q2,c)] bf16 -> [128, 32, 512] f32; q=(s2,o)=q2*128+qp."""
    arr = np.asarray(yo, np.float32).reshape(BC, 128, 2, 64)   # b, qp, q2, c
    arr = arr.transpose(0, 2, 1, 3).reshape(BC, 256, 64)       # b, q, c
    arr = arr.reshape(BC, NS, COUT, NCELL).transpose(0, 2, 3, 1)  # b,o,c,s2
    return np.ascontiguousarray(arr).reshape(BC, COUT, G)


def host_simulate(x, kern, bias, mapping):
    """f64 numpy mirror of the device algebra (layout validation)."""
    Cf, Ci, singles, reps = _transforms()
    Kexp = kern[:, :, mapping.reshape(NS, NS, NCELL)]
    KF = np.fft.fft2(
        Kexp.reshape(COUT, CIN, NS, NS, 8, 8).astype(np.float64), axes=(-2, -1)
    ).reshape(COUT, CIN, NS, NS, NCELL)
    xs = x.reshape(B, CIN, NCELL, NS).astype(np.float64)
    XF = np.einsum("bics,cf->bisf", xs, Cf)  # [b,i,s1,fc]
    yf = np.zeros((B, NS, COUT, 64))  # [b,s2,o,fc]
    for j, f in enumerate(reps):
        A = KF[..., f]
        yf[..., 4 + 2 * j] = (
            np.einsum("bis,oist->bto", XF[..., 4 + 2 * j], A.real)
            - np.einsum("bis,oist->bto", XF[..., 5 + 2 * j], A.imag)
        )
        yf[..., 5 + 2 * j] = (
            np.einsum("bis,oist->bto", XF[..., 4 + 2 * j], A.imag)
            + np.einsum("bis,oist->bto", XF[..., 5 + 2 * j], A.real)
        )
    for js, f in enumerate(singles):
        yf[..., js] = np.einsum("bis,oist->bto", XF[..., js], KF[..., f].real)
    yf[..., 0] += 64.0 * bias.ravel()[None, None, :]
    y = np.einsum("btof,fc->btoc", yf, Ci)  # [b,s2,o,c]
    y = y.transpose(0, 2, 3, 1).reshape(B, COUT, G)
    return y.astype(np.float32)


def _build_program():
    import concourse.bass as bass
    import concourse.bacc as bacc
    import concourse.mybir as mybir
    from concourse.tile import TileContext

    BF = mybir.dt.bfloat16
    F8 = mybir.dt.float8e4
    F32 = mybir.dt.float32
    nc = bacc.Bacc("TRN2", target_bir_lowering=False, debug=False,
                   num_devices=N_CORES)
    x_d = nc.dram_tensor("x", [128, 16384], BF, kind="ExternalInput")
    cfk_d = nc.dram_tensor("CfK", [128, 128], BF, kind="ExternalInput")
    cik_d = nc.dram_tensor("CiK", [128, 128], BF, kind="ExternalInput")
    wp_d = nc.dram_tensor("Wp", [4, 128, 8192], BF, kind="ExternalInput")
    ws_d = nc.dram_tensor("Ws", [128, 2048], BF, kind="ExternalInput")
    br_d = nc.dram_tensor("bias_row", [1, 256], BF, kind="ExternalInput")
    on_d = nc.dram_tensor("ones1", [1, 128], BF, kind="ExternalInput")
    id_d = nc.dram_tensor("ident", [128, 128], BF, kind="ExternalInput")
    y_d = nc.dram_tensor("y", [128, 16384], BF, kind="ExternalOutput")

    with TileContext(nc) as tc:
        with (
            tc.tile_pool(name="const", bufs=1) as cpool,
            tc.tile_pool(name="xt", bufs=1) as xtpool,
            tc.tile_pool(name="xf3", bufs=1) as xfpool,
            tc.tile_pool(name="w", bufs=1) as wpool,
            tc.tile_pool(name="yf", bufs=1) as yfpool,
            tc.tile_pool(name="yo", bufs=1) as yopool,
            tc.tile_pool(name="pbc", bufs=3) as pbcpool,
            tc.tile_pool(name="yt", bufs=3) as ytpool,
            tc.tile_pool(name="ps", bufs=2, space="PSUM") as pspool,
        ):
            cfk = cpool.tile([128, 128], BF, name="cfk")
            nc.sync.dma_start(out=cfk[:, :], in_=cfk_d[:, :])
            cik = cpool.tile([128, 128], BF, name="cik")
            nc.sync.dma_start(out=cik[:, :], in_=cik_d[:, :])
            br = cpool.tile([1, 256], BF, name="br")
            nc.sync.dma_start(out=br[:, :], in_=br_d[:, :])
            on = cpool.tile([1, 128], BF, name="on")
            nc.sync.dma_start(out=on[:, :], in_=on_d[:, :])
            ident = cpool.tile([128, 128], BF, name="ident")
            nc.sync.dma_start(out=ident[:, :], in_=id_d[:, :])

            # ALL input DMAs on the sync ring: one ring still uses all 16
            # SDMA engines per transfer, and issuing from the ACT (scalar)
            # ring was found to block the scalar sequencer's copies until
            # the whole in-stream drained. x first (C gates on it), then
            # weights in 1MB halves so stage D unblocks early.
            xt = [xtpool.tile([128, 2048], BF, name=f"xt{g}", tag=f"xt{g}")
                  for g in range(8)]
            for g in range(8):
                nc.sync.dma_start(
                    out=xt[g][:, :], in_=x_d[:, 2048 * g: 2048 * (g + 1)]
                )
            wsb = [wpool.tile([128, 8192], BF, name=f"wp{k}", tag=f"wp{k}")
                   for k in range(4)]
            for k in range(4):
                for hf in range(2):
                    nc.sync.dma_start(
                        out=wsb[k][:, 4096 * hf: 4096 * hf + 4096],
                        in_=wp_d[k][:, 4096 * hf: 4096 * hf + 4096],
                    )
            ws = wpool.tile([128, 2048], BF, name="ws", tag="ws")
            nc.sync.dma_start(out=ws[:, :], in_=ws_d[:, :])

            xf3 = xfpool.tile([128, 16384], BF, name="xf3")
            yf = yfpool.tile([128, 16384], BF, name="yf")
            yo = yopool.tile([128, 16384], BF, name="yo")

            # ---- stage C: forward cell-DFT, one matmul per batch ----
            # rotate psum across all four same-size tags for a deep pipeline
            ptags = ["psA", "pa", "pb", "psT"]
            for t in range(32):
                g, tl = divmod(t, 4)
                xtr = xt[g][:, :].rearrange(
                    "p (t b4 r) -> p t b4 r", t=4, b4=4
                )
                pc = pspool.tile([128, 512], F32, name="pc", tag=ptags[t % 4])
                for b4 in range(4):
                    nc.tensor.matmul(
                        pc[:, 128 * b4: 128 * b4 + 128],
                        xtr[:, tl, b4, :],
                        cfk[:, :],
                        start=True, stop=True,
                    )
                dst = xf3[:, :].rearrange("p (t q) -> p t q", t=32)
                if t % 2:
                    nc.scalar.copy(dst[:, t, :], pc[:, :])
                else:
                    nc.vector.tensor_copy(dst[:, t, :], pc[:, :])

            # ---- stage D: per-frequency mixing ----
            # yf free = (m, qp) with m = q2*64+fc, q = q2*128+qp: combine
            # writes land as two contiguous 128-runs, and stage-E chunk qp
            # is a single stride-128 run (legal stationary-operand AP).
            xf3r = xf3[:, :].rearrange(
                "p (b s2 fc) -> p s2 fc b", s2=2, fc=64
            )
            yfm = yf[:, :].rearrange(
                "p (q2 fc qp) -> p q2 fc qp", q2=2, fc=64
            )
            for j in range(30):
                k, unit0 = divmod(2 * j, 16)
                ta, tb = ("pa", "pb") if j % 2 == 0 else ("psA", "psT")
                pa = pspool.tile([128, 512], F32, name="pa", tag=ta)
                pb = pspool.tile([128, 512], F32, name="pb", tag=tb)
                for h in range(2):
                    rhs = wsb[k][:, 512 * (unit0 + h): 512 * (unit0 + h) + 512]
                    nc.tensor.matmul(
                        pb[:, :], xf3r[:, h, 5 + 2 * j, :], rhs,
                        start=(h == 0), stop=(h == 1),
                    )
                pbc = pbcpool.tile([128, 512], BF, name="pbc", tag="pbc")
                nc.scalar.copy(pbc[:, :], pb[:, :])
                for h in range(2):
                    rhs = wsb[k][:, 512 * (unit0 + h): 512 * (unit0 + h) + 512]
                    nc.tensor.matmul(
                        pa[:, :], xf3r[:, h, 4 + 2 * j, :], rhs,
                        start=(h == 0), stop=(h == 1),
                    )
                fr, fi = 4 + 2 * j, 5 + 2 * j
                par = pa[:, :].rearrange("p (ri q2 qp) -> p ri q2 qp", ri=2, q2=2)
                pbr = pbc[:, :].rearrange("p (ri q2 qp) -> p ri q2 qp", ri=2, q2=2)
                nc.vector.tensor_sub(
                    yfm[:, :, fr, :], par[:, 0], pbr[:, 1]
                )
                nc.vector.tensor_add(
                    yfm[:, :, fi, :], par[:, 1], pbr[:, 0]
                )
            for js in range(4):
                pa = pspool.tile([128, 512], F32, name="pas", tag=ptags[js])
                for h in range(2):
                    rhs = ws[:, 256 * (2 * js + h): 256 * (2 * js + h) + 256]
                    nc.tensor.matmul(
                        pa[:, 0:256], xf3r[:, h, js, :], rhs,
                        start=(h == 0), stop=(h == 1 and js != 0),
                    )
                if js == 0:
                    nc.tensor.matmul(
                        pa[:, 0:256], on[:, :], br[:, :],
                        start=False, stop=True,
                    )
                pasr = pa[:, 0:256].rearrange("p (q2 qp) -> p q2 qp", q2=2)
                nc.vector.tensor_copy(yfm[:, :, js, :], pasr[:, :, :])

            # ---- stage E: PE transpose + inverse cell-DFT ----
            # chunk qp = cols {qp + 128*m'}: single stride-128 run
            yfq = yf[:, :].rearrange("p (m qp) -> p qp m", m=128)
            pe = None
            for grp in range(16):
                pt = pspool.tile([128, 1024], BF, name="pt", tag="psT")
                for k in range(8):
                    qp = 8 * grp + k
                    nc.tensor.transpose(
                        pt[:, 128 * k: 128 * k + 128],
                        yfq[:, qp, :],
                        ident[:, :],
                    )
                yt = ytpool.tile([128, 1024], BF, name="yt", tag="yt")
                # split the staging copy across both engines to halve the
                # latency in the transpose->matmul chain
                nc.scalar.copy(yt[:, 0:512], pt[:, 0:512])
                nc.vector.tensor_copy(yt[:, 512:1024], pt[:, 512:1024])
                for k in range(8):
                    qp = 8 * grp + k
                    if qp % 4 == 0:
                        pe = pspool.tile(
                            [128, 512], F32, name="pe",
                            tag=["psA", "pa", "pb"][(qp // 4) % 3],
                        )
                    nc.tensor.matmul(
                        pe[:, 128 * (qp % 4): 128 * (qp % 4) + 128],
                        yt[:, 128 * k: 128 * k + 128], cik[:, :],
                        start=True, stop=True,
                    )
                    if qp % 4 == 3:
                        quad = qp // 4
                        dst = yo[:, 512 * quad: 512 * quad + 512]
                        if quad % 2:
                            nc.scalar.copy(dst, pe[:, :])
                        else:
                            nc.vector.tensor_copy(dst, pe[:, :])
                if grp % 2 == 1:
                    blk = grp // 2
                    nc.sync.dma_start(
                        out=y_d[:, 2048 * blk: 2048 * blk + 2048],
                        in_=yo[:, 2048 * blk: 2048 * blk + 2048],
                    )
    nc.compile()
    return nc


def kernel(**inputs):
    x = np.asarray(inputs["x"], np.float32)
    kern = np.asarray(inputs["kernel"], np.float32)
    bias = np.asarray(inputs["bias"], np.float32)
    mapping = np.asarray(inputs["mapping"])
    from concourse.bass_utils import run_bass_kernel_spmd

    if "nc" not in _CACHE:
        _CACHE["nc"] = _build_program()
    nc = _CACHE["nc"]
    consts = host_constants(kern, bias, mapping)
    in_maps = []
    for c in range(N_CORES):
        m = dict(consts)
        m["x"] = host_prep_x(x[c * BC: (c + 1) * BC])
        in_maps.append(m)
    res = run_bass_kernel_spmd(nc, in_maps, list(range(N_CORES)))
    _CACHE["last_exec_ns"] = res.exec_time_ns
    y = np.concatenate(
        [host_unpack_y(res.results[c]["y"]) for c in range(N_CORES)], 0
    )
    return np.ascontiguousarray(y.astype(np.float32))


# revision 35
# speedup vs baseline: 1.4058x; 1.0296x over previous
"""DenseEquivariantFFT Trainium2 kernel (batch-sharded over 8 cores), v2.

Math: y = IDFT2_cells( sum_{i,s1} DFT2_cells(x)[b,i,s1,f] * KF[o,i,s1,s2,f] ) + bias
where KF = DFT2_cells(kernel[..,mapping]) and f runs over the 64 cell
frequencies in a real (cos/sin) basis.

Device dataflow per core (128 batches), all bf16 with f32 PSUM accum:
 - host pre-transposes x into [(s1-parity, cell), (t, b4, sp, i)] layout,
   so no on-device input transposes are needed.
 - stage C: per batch, one matmul [K=128=(par,c)] x blockdiag(Cf) -> XF
   with partitions (sp,i) and free (batch, parity, fc).
 - stage D: per frequency pair, 4 matmuls [K=128, N=512] against
   deduplicated [kr|ki] weights; re/im recombined on the vector engine
   into an fc-major yf (contiguous writes).
 - stage E: PE transposes (8 per PSUM bank) put (q2,fc) on partitions,
   then one matmul per q-pair against blockdiag(Ci) produces spatial
   output with batch back on partitions; host un-permutes the layout.
"""
import numpy as np
import ml_dtypes

N_CORES = 8
B, CIN, COUT, NS, NCELL, G = 1024, 32, 32, 8, 64, 512
BC = B // N_CORES  # 128 batches per core

_CACHE = {}


def _freq_classes():
    singles, reps = [], []
    for ku in range(8):
        for kv in range(8):
            f = ku * 8 + kv
            cf = ((-ku) % 8) * 8 + ((-kv) % 8)
            if cf == f:
                singles.append(f)
            elif f < cf:
                reps.append(f)
    return singles, reps  # 4, 30


def _transforms():
    singles, reps = _freq_classes()
    u, v = np.meshgrid(np.arange(8), np.arange(8), indexing="ij")

    def theta(f):
        ku, kv = divmod(f, 8)
        return 2 * np.pi * (ku * u + kv * v) / 8

    Cf = np.zeros((64, 64))
    Ci = np.zeros((64, 64))
    for j, f in enumerate(singles):
        Cf[:, j] = np.cos(theta(f)).ravel()
        Ci[j, :] = np.cos(theta(f)).ravel() / 64
    for j, f in enumerate(reps):
        Cf[:, 4 + 2 * j] = np.cos(theta(f)).ravel()
        Cf[:, 5 + 2 * j] = -np.sin(theta(f)).ravel()
        Ci[4 + 2 * j, :] = 2 * np.cos(theta(f)).ravel() / 64
        Ci[5 + 2 * j, :] = -2 * np.sin(theta(f)).ravel() / 64
    return Cf, Ci, singles, reps


def host_constants(kern, bias, mapping):
    """Device weight tensors. W rows use r=(sp,i) with s1=2*sp+h (parity
    halves); W cols use q=(s2,o)."""
    Cf, Ci, singles, reps = _transforms()
    Kexp = kern[:, :, mapping.reshape(NS, NS, NCELL)]  # [o,i,s1,s2,c]
    KF = np.fft.fft2(
        Kexp.reshape(COUT, CIN, NS, NS, 8, 8).astype(np.float64), axes=(-2, -1)
    ).reshape(COUT, CIN, NS, NS, NCELL)

    wp = np.zeros((64, 128, 512), np.float64)  # unit = 2*j + h
    for j, f in enumerate(reps):
        A = KF[..., f]  # [o,i,s1,s2]
        krf = A.real.transpose(2, 1, 3, 0).reshape(NS, CIN, NS * COUT)
        kif = A.imag.transpose(2, 1, 3, 0).reshape(NS, CIN, NS * COUT)
        for h in range(2):
            kr = krf[h::2].reshape(128, 256)
            ki = kif[h::2].reshape(128, 256)
            wp[2 * j + h] = np.concatenate([kr, ki], axis=1)
    ws = np.zeros((8, 128, 256), np.float64)  # unit = 2*js + h
    for js, f in enumerate(singles):
        A = KF[..., f].real.transpose(2, 1, 3, 0).reshape(NS, CIN, NS * COUT)
        for h in range(2):
            ws[2 * js + h] = A[h::2].reshape(128, 256)

    bias_row = 64.0 * np.tile(bias.ravel().astype(np.float64), NS)[None, :]
    bf = ml_dtypes.bfloat16
    return {
        "CfK": np.kron(np.eye(2), Cf).astype(bf),          # [128,128]
        "CiK": np.kron(np.eye(2), Ci).astype(bf),          # [128,128]
        "Wp": np.ascontiguousarray(
            wp.reshape(4, 16, 128, 512).transpose(0, 2, 1, 3)
        ).reshape(4, 128, 16 * 512).astype(bf),
        "Ws": np.ascontiguousarray(
            ws.transpose(1, 0, 2)
        ).reshape(128, 8 * 256).astype(bf),
        "bias_row": bias_row.astype(bf),
        "ones1": np.ones((1, 128), bf),
        "ident": np.eye(128).astype(bf),
    }


def host_prep_x(xc):
    """[128,32,512] f32 -> [128=(par,c), 16384=(t,b4,sp,i)] bf16."""
    xs = xc.reshape(32, 4, CIN, NCELL, 4, 2)  # t,b4,i,c,sp,par
    xt2 = xs.transpose(5, 3, 0, 1, 4, 2).reshape(128, 16384)
    return np.ascontiguousarray(xt2.astype(ml_dtypes.bfloat16))


def host_unpack_y(yo):
    """[128, 16384=(qp,q2,c)] bf16 -> [128, 32, 512] f32; q=(s2,o)=q2*128+qp."""
    arr = np.asarray(yo, np.float32).reshape(BC, 128, 2, 64)   # b, qp, q2, c
    arr = arr.transpose(0, 2, 1, 3).reshape(BC, 256, 64)       # b, q, c
    arr = arr.reshape(BC, NS, COUT, NCELL).transpose(0, 2, 3, 1)  # b,o,c,s2
    return np.ascontiguousarray(arr).reshape(BC, COUT, G)


def host_simulate(x, kern, bias, mapping):
    """f64 numpy mirror of the device algebra (layout validation)."""
    Cf, Ci, singles, reps = _transforms()
    Kexp = kern[:, :, mapping.reshape(NS, NS, NCELL)]
    KF = np.fft.fft2(
        Kexp.reshape(COUT, CIN, NS, NS, 8, 8).astype(np.float64), axes=(-2, -1)
    ).reshape(COUT, CIN, NS, NS, NCELL)
    xs = x.reshape(B, CIN, NCELL, NS).astype(np.float64)
    XF = np.einsum("bics,cf->bisf", xs, Cf)  # [b,i,s1,fc]
    yf = np.zeros((B, NS, COUT, 64))  # [b,s2,o,fc]
    for j, f in enumerate(reps):
        A = KF[..., f]
        yf[..., 4 + 2 * j] = (
            np.einsum("bis,oist->bto", XF[..., 4 + 2 * j], A.real)
            - np.einsum("bis,oist->bto", XF[..., 5 + 2 * j], A.imag)
        )
        yf[..., 5 + 2 * j] = (
            np.einsum("bis,oist->bto", XF[..., 4 + 2 * j], A.imag)
            + np.einsum("bis,oist->bto", XF[..., 5 + 2 * j], A.real)
        )
    for js, f in enumerate(singles):
        yf[..., js] = np.einsum("bis,oist->bto", XF[..., js], KF[..., f].real)
    yf[..., 0] += 64.0 * bias.ravel()[None, None, :]
    y = np.einsum("btof,fc->btoc", yf, Ci)  # [b,s2,o,c]
    y = y.transpose(0, 2, 3, 1).reshape(B, COUT, G)
    return y.astype(np.float32)


def _build_program():
    import concourse.bass as bass
    import concourse.bacc as bacc
    import concourse.mybir as mybir
    from concourse.tile import TileContext

    BF = mybir.dt.bfloat16
    F8 = mybir.dt.float8e4
    F32 = mybir.dt.float32
    nc = bacc.Bacc("TRN2", target_bir_lowering=False, debug=False,
                   num_devices=N_CORES)
    x_d = nc.dram_tensor("x", [128, 16384], BF, kind="ExternalInput")
    cfk_d = nc.dram_tensor("CfK", [128, 128], BF, kind="ExternalInput")
    cik_d = nc.dram_tensor("CiK", [128, 128], BF, kind="ExternalInput")
    wp_d = nc.dram_tensor("Wp", [4, 128, 8192], BF, kind="ExternalInput")
    ws_d = nc.dram_tensor("Ws", [128, 2048], BF, kind="ExternalInput")
    br_d = nc.dram_tensor("bias_row", [1, 256], BF, kind="ExternalInput")
    on_d = nc.dram_tensor("ones1", [1, 128], BF, kind="ExternalInput")
    id_d = nc.dram_tensor("ident", [128, 128], BF, kind="ExternalInput")
    y_d = nc.dram_tensor("y", [128, 16384], BF, kind="ExternalOutput")

    with TileContext(nc) as tc:
        with (
            tc.tile_pool(name="const", bufs=1) as cpool,
            tc.tile_pool(name="xt", bufs=1) as xtpool,
            tc.tile_pool(name="xf3", bufs=1) as xfpool,
            tc.tile_pool(name="w", bufs=1) as wpool,
            tc.tile_pool(name="yf", bufs=1) as yfpool,
            tc.tile_pool(name="yo", bufs=1) as yopool,
            tc.tile_pool(name="pbc", bufs=3) as pbcpool,
            tc.tile_pool(name="yt", bufs=3) as ytpool,
            tc.tile_pool(name="ps", bufs=2, space="PSUM") as pspool,
        ):
            cfk = cpool.tile([128, 128], BF, name="cfk")
            nc.sync.dma_start(out=cfk[:, :], in_=cfk_d[:, :])
            cik = cpool.tile([128, 128], BF, name="cik")
            nc.sync.dma_start(out=cik[:, :], in_=cik_d[:, :])
            br = cpool.tile([1, 256], BF, name="br")
            nc.sync.dma_start(out=br[:, :], in_=br_d[:, :])
            on = cpool.tile([1, 128], BF, name="on")
            nc.sync.dma_start(out=on[:, :], in_=on_d[:, :])
            ident = cpool.tile([128, 128], BF, name="ident")
            nc.sync.dma_start(out=ident[:, :], in_=id_d[:, :])

            # ALL input DMAs on the sync ring: one ring still uses all 16
            # SDMA engines per transfer, and issuing from the ACT (scalar)
            # ring was found to block the scalar sequencer's copies until
            # the whole in-stream drained. x first (C gates on it), then
            # weights in 1MB halves so stage D unblocks early.
            # first chunk small so stage C starts ASAP; later chunks big
            # enough to keep DMA efficiency up. Sizes in t-units (512 cols).
            xchunks = [2, 6, 4, 4, 4, 4, 4, 4]
            xoff = [0]
            for n in xchunks:
                xoff.append(xoff[-1] + n)
            xt = [xtpool.tile([128, 512 * xchunks[g]], BF,
                              name=f"xt{g}", tag=f"xt{g}")
                  for g in range(8)]
            for g in range(8):
                nc.sync.dma_start(
                    out=xt[g][:, :],
                    in_=x_d[:, 512 * xoff[g]: 512 * xoff[g + 1]],
                )
            wsb = [wpool.tile([128, 8192], BF, name=f"wp{k}", tag=f"wp{k}")
                   for k in range(4)]
            for k in range(4):
                for hf in range(2):
                    nc.sync.dma_start(
                        out=wsb[k][:, 4096 * hf: 4096 * hf + 4096],
                        in_=wp_d[k][:, 4096 * hf: 4096 * hf + 4096],
                    )
            ws = wpool.tile([128, 2048], BF, name="ws", tag="ws")
            nc.sync.dma_start(out=ws[:, :], in_=ws_d[:, :])

            xf3 = xfpool.tile([128, 16384], BF, name="xf3")
            yf = yfpool.tile([128, 16384], BF, name="yf")
            yo = yopool.tile([128, 16384], BF, name="yo")

            # ---- stage C: forward cell-DFT, one matmul per batch ----
            # rotate psum across all four same-size tags for a deep pipeline
            ptags = ["psA", "pa", "pb", "psT"]
            for t in range(32):
                g = next(i for i in range(8) if xoff[i + 1] > t)
                tl = t - xoff[g]
                xtr = xt[g][:, :].rearrange(
                    "p (t b4 r) -> p t b4 r", t=xchunks[g], b4=4
                )
                pc = pspool.tile([128, 512], F32, name="pc", tag=ptags[t % 4])
                for b4 in range(4):
                    nc.tensor.matmul(
                        pc[:, 128 * b4: 128 * b4 + 128],
                        xtr[:, tl, b4, :],
                        cfk[:, :],
                        start=True, stop=True,
                    )
                dst = xf3[:, :].rearrange("p (t q) -> p t q", t=32)
                if t % 2:
                    nc.scalar.copy(dst[:, t, :], pc[:, :])
                else:
                    nc.vector.tensor_copy(dst[:, t, :], pc[:, :])

            # ---- stage D: per-frequency mixing ----
            # yf free = (m, qp) with m = q2*64+fc, q = q2*128+qp: combine
            # writes land as two contiguous 128-runs, and stage-E chunk qp
            # is a single stride-128 run (legal stationary-operand AP).
            xf3r = xf3[:, :].rearrange(
                "p (b s2 fc) -> p s2 fc b", s2=2, fc=64
            )
            yfm = yf[:, :].rearrange(
                "p (q2 fc qp) -> p q2 fc qp", q2=2, fc=64
            )
            for j in range(30):
                k, unit0 = divmod(2 * j, 16)
                ta, tb = ("pa", "pb") if j % 2 == 0 else ("psA", "psT")
                pa = pspool.tile([128, 512], F32, name="pa", tag=ta)
                pb = pspool.tile([128, 512], F32, name="pb", tag=tb)
                for h in range(2):
                    rhs = wsb[k][:, 512 * (unit0 + h): 512 * (unit0 + h) + 512]
                    nc.tensor.matmul(
                        pb[:, :], xf3r[:, h, 5 + 2 * j, :], rhs,
                        start=(h == 0), stop=(h == 1),
                    )
                pbc = pbcpool.tile([128, 512], BF, name="pbc", tag="pbc")
                nc.scalar.copy(pbc[:, :], pb[:, :])
                for h in range(2):
                    rhs = wsb[k][:, 512 * (unit0 + h): 512 * (unit0 + h) + 512]
                    nc.tensor.matmul(
                        pa[:, :], xf3r[:, h, 4 + 2 * j, :], rhs,
                        start=(h == 0), stop=(h == 1),
                    )
                fr, fi = 4 + 2 * j, 5 + 2 * j
                par = pa[:, :].rearrange("p (ri q2 qp) -> p ri q2 qp", ri=2, q2=2)
                pbr = pbc[:, :].rearrange("p (ri q2 qp) -> p ri q2 qp", ri=2, q2=2)
                nc.vector.tensor_sub(
                    yfm[:, :, fr, :], par[:, 0], pbr[:, 1]
                )
                nc.vector.tensor_add(
                    yfm[:, :, fi, :], par[:, 1], pbr[:, 0]
                )
            for js in range(4):
                pa = pspool.tile([128, 512], F32, name="pas", tag=ptags[js])
                for h in range(2):
                    rhs = ws[:, 256 * (2 * js + h): 256 * (2 * js + h) + 256]
                    nc.tensor.matmul(
                        pa[:, 0:256], xf3r[:, h, js, :], rhs,
                        start=(h == 0), stop=(h == 1 and js != 0),
                    )
                if js == 0:
                    nc.tensor.matmul(
                        pa[:, 0:256], on[:, :], br[:, :],
                        start=False, stop=True,
                    )
                pasr = pa[:, 0:256].rearrange("p (q2 qp) -> p q2 qp", q2=2)
                nc.vector.tensor_copy(yfm[:, :, js, :], pasr[:, :, :])

            # ---- stage E: PE transpose + inverse cell-DFT ----
            # chunk qp = cols {qp + 128*m'}: single stride-128 run
            yfq = yf[:, :].rearrange("p (m qp) -> p qp m", m=128)
            pe = None
            for grp in range(16):
                pt = pspool.tile([128, 1024], BF, name="pt", tag="psT")
                for k in range(8):
                    qp = 8 * grp + k
                    nc.tensor.transpose(
                        pt[:, 128 * k: 128 * k + 128],
                        yfq[:, qp, :],
                        ident[:, :],
                    )
                yt = ytpool.tile([128, 1024], BF, name="yt", tag="yt")
                # split the staging copy across both engines to halve the
                # latency in the transpose->matmul chain
                nc.scalar.copy(yt[:, 0:512], pt[:, 0:512])
                nc.vector.tensor_copy(yt[:, 512:1024], pt[:, 512:1024])
                for k in range(8):
                    qp = 8 * grp + k
                    if qp % 4 == 0:
                        pe = pspool.tile(
                            [128, 512], F32, name="pe",
                            tag=["psA", "pa", "pb"][(qp // 4) % 3],
                        )
                    nc.tensor.matmul(
                        pe[:, 128 * (qp % 4): 128 * (qp % 4) + 128],
                        yt[:, 128 * k: 128 * k + 128], cik[:, :],
                        start=True, stop=True,
                    )
                    if qp % 4 == 3:
                        quad = qp // 4
                        dst = yo[:, 512 * quad: 512 * quad + 512]
                        if quad % 2:
                            nc.scalar.copy(dst, pe[:, :])
                        else:
                            nc.vector.tensor_copy(dst, pe[:, :])
                nc.sync.dma_start(
                    out=y_d[:, 1024 * grp: 1024 * grp + 1024],
                    in_=yo[:, 1024 * grp: 1024 * grp + 1024],
                )
    nc.compile()
    return nc


def kernel(**inputs):
    x = np.asarray(inputs["x"], np.float32)
    kern = np.asarray(inputs["kernel"], np.float32)
    bias = np.asarray(inputs["bias"], np.float32)
    mapping = np.asarray(inputs["mapping"])
    from concourse.bass_utils import run_bass_kernel_spmd

    if "nc" not in _CACHE:
        _CACHE["nc"] = _build_program()
    nc = _CACHE["nc"]
    consts = host_constants(kern, bias, mapping)
    in_maps = []
    for c in range(N_CORES):
        m = dict(consts)
        m["x"] = host_prep_x(x[c * BC: (c + 1) * BC])
        in_maps.append(m)
    res = run_bass_kernel_spmd(nc, in_maps, list(range(N_CORES)))
    _CACHE["last_exec_ns"] = res.exec_time_ns
    y = np.concatenate(
        [host_unpack_y(res.results[c]["y"]) for c in range(N_CORES)], 0
    )
    return np.ascontiguousarray(y.astype(np.float32))
